# revision 18
# baseline (speedup 1.0000x reference)
"""BERT-base (12-layer, B=8, S=512, D=768, H=12, F=3072) forward pass on 8
Trainium2 NeuronCores — v2.

Strategy: data-parallel over batch (1 sequence per core, no collectives).
Key structure (per core, activations feature-major xT[D, S] in SBUF):
  - the FFN in this model is LINEAR (no activation between W1/W2), so
    W2@W1 collapses on the host into one 768x768 matrix Weff; the attn-LN
    affine (g,b) folds into Weff/beff as well, and the LN normalize
    (mean/rstd) folds into the GEMM via a rank-1 correction + per-column
    scale at PSUM evacuation. The FFN costs 1/4 of the naive FLOPs and
    layer-norm #1 never stalls the PE.
  - softmax denominators fall out of a ones-augmented V column (PV matmul
    M=65); reciprocals via Ln/Exp on the denominator row (partition 64).
  - weights are host-repacked so every weight DMA is contiguous per
    partition (1.5-6KB lines instead of 256B).
  - emission interleaves Q/K projections with scores+exp per head-pair so
    the ACT-bound softmax overlaps PE GEMM work; dummy warm matmuls keep
    the PE HAM clock at 2.4GHz across LN chains.
"""
import numpy as np

import concourse.bass as bass
import concourse.mybir as mybir
import concourse.tile as tile
from concourse import bass_utils
from concourse.masks import make_identity

AF = mybir.ActivationFunctionType
OP = mybir.AluOpType
F32 = mybir.dt.float32
F32R = mybir.dt.float32r
BF16 = mybir.dt.bfloat16
I32 = mybir.dt.int32

B, S, D, H, F, L, V = 8, 512, 768, 12, 3072, 12, 30522
DK = D // H
SCALE = 1.0 / float(np.sqrt(DK))
NT = D // 128      # 6 feature tiles
NST = S // 128     # 4 sequence tiles
NP = H // 2        # 6 head pairs

_NC_CACHE = None


# ---------------------------------------------------------------------------
# wait-slot legalization: walrus codegen allows only ONE sync-wait command on
# TPB instructions; hoist excess waits into standalone EventSemaphores.
def _legalize_waits(nc):
    skip = (mybir.InstEventSemaphore, mybir.InstNoOp)
    n = 0
    for fn in nc.m.functions:
        for blk in fn.blocks:
            out = []
            for inst in blk.instructions:
                si = inst.sync_info
                if si is not None and si.on_wait and not isinstance(inst, skip) \
                        and len(si.on_wait) > 1:
                    waits = list(si.on_wait)
                    for j, w in enumerate(waits[:-1]):
                        ev = mybir.InstEventSemaphore(
                            name=f"{inst.name}-lgw{j}", ins=[], outs=[],
                            sync_info=mybir.SyncInfo(on_wait=[w], on_update=[]),
                        )
                        ev.engine = inst.engine
                        out.append(ev)
                        n += 1
                    inst.sync_info = mybir.SyncInfo(
                        on_wait=[waits[-1]], on_update=list(si.on_update))
                out.append(inst)
            try:
                blk.instructions = out
            except Exception:
                blk.instructions.clear()
                blk.instructions.extend(out)
    return n


def _build_nc():
    nc = bass.Bass("TRN2", target_bir_lowering=False, debug=False,
                   enable_asserts=False, num_devices=8)

    # ---- DRAM I/O ---------------------------------------------------------
    d = {}
    d["d_ids"] = nc.dram_tensor("ids", [S, 1], I32, kind="ExternalInput")
    d["d_tti"] = nc.dram_tensor("tti", [S, 1], I32, kind="ExternalInput")
    d["d_mask"] = nc.dram_tensor("maskadd", [S], F32, kind="ExternalInput")
    d["d_wemb"] = nc.dram_tensor("wemb", [V, D], F32, kind="ExternalInput")
    d["d_pemb"] = nc.dram_tensor("pemb", [S, D], F32, kind="ExternalInput")
    d["d_temb"] = nc.dram_tensor("temb", [2, D], F32, kind="ExternalInput")

    # repacked weights: [L, et, p, n, e] so each (l, et) chunk is contiguous
    d["d_wq"] = nc.dram_tensor("wq", [L, NT, 128, NT, 128], BF16, kind="ExternalInput")
    d["d_wk"] = nc.dram_tensor("wk", [L, NT, 128, NT, 128], BF16, kind="ExternalInput")
    d["d_wo"] = nc.dram_tensor("wo", [L, NT, 128, NT, 128], BF16, kind="ExternalInput")
    d["d_wf"] = nc.dram_tensor("wf", [L, NT, 128, NT, 128], F32, kind="ExternalInput")
    d["d_wva"] = nc.dram_tensor("wva", [L, 128, NT, 512], BF16, kind="ExternalInput")
    d["d_wvb"] = nc.dram_tensor("wvb", [L, 128, NT, 256], BF16, kind="ExternalInput")
    # packed per-layer params: columns [128, 30] and rows [1, 3*768]
    d["d_cols"] = nc.dram_tensor("cols", [L, 128, 30], F32, kind="ExternalInput")
    d["d_rows"] = nc.dram_tensor("rows", [L, 4 * D], F32, kind="ExternalInput")
    d["d_egr"] = nc.dram_tensor("egr", [1, D], F32, kind="ExternalInput")
    d["d_ones"] = nc.dram_tensor("ones128", [128], F32, kind="ExternalInput")
    d["d_onesb"] = nc.dram_tensor("ones128b", [128], BF16, kind="ExternalInput")
    d["d_ones512"] = nc.dram_tensor("ones512", [1, 512], F32, kind="ExternalInput")
    d["d_neg1"] = nc.dram_tensor("neg1", [1, 128], F32, kind="ExternalInput")
    d["d_selA"] = nc.dram_tensor("selA", [1, 128], F32, kind="ExternalInput")
    d["d_selB"] = nc.dram_tensor("selB", [1, 128], F32, kind="ExternalInput")
    d["d_onesgb"] = nc.dram_tensor("onesgridb", [128, NST * H], BF16, kind="ExternalInput")
    d["d_out"] = nc.dram_tensor("out", [S, D], F32, kind="ExternalOutput")

    with tile.TileContext(nc) as tc:
        _emit(nc, tc, d)
    _legalize_waits(nc)
    return nc


def _emit(nc, tc, d):
    import contextlib
    ctx = contextlib.ExitStack()
    with ctx:
        _emit_body(nc, tc, d, ctx)


def _emit_body(nc, tc, d, ctx):
    pool = ctx.enter_context(tc.tile_pool(name="persist", bufs=1))
    wqp = ctx.enter_context(tc.tile_pool(name="wqp", bufs=2))
    wkp = ctx.enter_context(tc.tile_pool(name="wkp", bufs=2))
    wop = ctx.enter_context(tc.tile_pool(name="wop", bufs=4))
    wfp = ctx.enter_context(tc.tile_pool(name="wfp", bufs=4))
    vwp = ctx.enter_context(tc.tile_pool(name="vwp", bufs=2))
    ppool = ctx.enter_context(tc.tile_pool(name="params", bufs=2))
    rpool = ctx.enter_context(tc.tile_pool(name="rowsp", bufs=1))
    epool = ctx.enter_context(tc.tile_pool(name="epool", bufs=10))
    spool = ctx.enter_context(tc.tile_pool(name="smalls", bufs=1))
    sqpool = ctx.enter_context(tc.tile_pool(name="sqp", bufs=1))

    # ---- persistent constants --------------------------------------------
    ones_col = pool.tile([128, 1], F32R, name="ones_col")
    nc.sync.dma_start(ones_col[:], d["d_ones"].ap().rearrange("(p o) -> p o", o=1).bitcast(F32R))
    ones_colb = pool.tile([128, 1], BF16, name="ones_colb")
    nc.sync.dma_start(ones_colb[:], d["d_onesb"].ap().rearrange("(p o) -> p o", o=1))
    one_row = pool.tile([1, 128], F32R, name="one_row")
    nc.sync.dma_start(one_row[:], d["d_ones"].ap().rearrange("(o p) -> o p", o=1).bitcast(F32R))
    ones_s = pool.tile([1, 512], F32R, name="ones_s")
    nc.sync.dma_start(ones_s[:], d["d_ones512"].ap()[:, :].bitcast(F32R))
    neg_row = pool.tile([1, 128], F32R, name="neg_row")
    nc.sync.dma_start(neg_row[:], d["d_neg1"].ap()[:, :].bitcast(F32R))
    # head-select rows living on partition 64 (same partition as the PV
    # denominator row) so the broadcast matmul's operands share a base.
    sel64 = pool.tile([65, 2, 128], F32R, name="sel64")
    nc.sync.dma_start(sel64[64:65, 0, :], d["d_selA"].ap()[:, :].bitcast(F32R))
    nc.sync.dma_start(sel64[64:65, 1, :], d["d_selB"].ap()[:, :].bitcast(F32R))
    ident = pool.tile([128, 128], F32, name="ident")
    make_identity(nc, ident[:])
    ident16 = pool.tile([128, 128], BF16, name="ident16")
    make_identity(nc, ident16[:])
    maskc = pool.tile([128, NST], F32, name="maskc")
    nc.sync.dma_start(maskc[:], d["d_mask"].ap().rearrange("(n p) -> p n", p=128))

    # ---- persistent activations ------------------------------------------
    xT = pool.tile([128, NT, S], BF16, name="xT")       # layer input, feature-major
    qT = pool.tile([128, NT, S], BF16, name="qT")
    kT = pool.tile([128, NT, S], BF16, name="kT")
    cT = pool.tile([128, NT, S], BF16, name="cT")       # ctx, feature-major
    ybuf = pool.tile([128, NT, S], F32R, name="ybuf")   # post-Wo residual
    ybuf2 = pool.tile([128, NT, S], F32R, name="ybuf2")  # post-FFN (pre-LN2)
    vaug = pool.tile([128, NST, H, DK + 1], BF16, name="vaug")
    nc.sync.dma_start(
        vaug[:, :, :, DK:DK + 1],
        d["d_onesgb"].ap().rearrange("p (a b) -> p a b", a=NST)[:, :, :],
    )

    def warm_mm(wps):
        t = wps.tile([128, S], F32, name="warm", tag="warm")
        nc.tensor.matmul(t[:], one_row[:], ones_s[:], start=True, stop=True)

    # =======================================================================
    # folded layernorm (embedding LN + LN2): y [128, nt, S] F32R ->
    # out = (y - mu) * rstd * g  (gamma via rank-1 g x rstd broadcasts; beta
    # is host-folded into downstream biases, except `bias_col` for the last
    # layer). sq_t are precomputed squares of y. Dummy warm matmuls chained
    # on each output tile keep the PE HAM clock warm across the DVE tail.
    def layernorm_fold(y, sq_t, g_rows, g_off, eps, out, psum_pool, rgp,
                       bias_col=None):
        s0 = psum_pool.tile([1, S], F32, name="s0", tag="st0")
        s1t = psum_pool.tile([33, S], F32, name="s1t", tag="st1")
        # col-packed stats: s0 -> col group 0, s1 -> col group 1 (concurrent)
        for dt in range(NT):
            nc.tensor.matmul(s0[:], ones_col[:], y[:, dt, :],
                             start=(dt == 0), stop=(dt == NT - 1))
            nc.tensor.matmul(s1t[32:33, :], ones_colb[:], sq_t[dt][:],
                             start=(dt == 0), stop=(dt == NT - 1))
        mu = spool.tile([1, S], F32R, name="mu", tag="ln_mu")
        nc.vector.tensor_scalar(mu[:], s0[:], 1.0 / D, None, OP.mult)
        msq = spool.tile([1, S], F32, name="msq", tag="ln_msq")
        nc.vector.tensor_scalar(msq[:], s1t[32:33, :], 1.0 / D, eps, OP.mult, OP.add)
        musq = spool.tile([1, S], F32, name="musq", tag="ln_musq")
        nc.vector.tensor_tensor(musq[:], mu[:].bitcast(F32), mu[:].bitcast(F32), op=OP.mult)
        var = spool.tile([1, S], F32R, name="var", tag="ln_var")
        nc.vector.tensor_tensor(var[:], msq[:], musq[:], op=OP.subtract)
        # warm keeper chained on var: bridges the stats->rstd ACT stretch
        wv_ = psum_pool.tile([128, S], F32, name="wv", tag="warm")
        nc.tensor.matmul(wv_[:], one_row[:], var[:], start=True, stop=True)
        lnv = spool.tile([1, S], F32, name="lnv", tag="ln_lnv")
        nc.scalar.activation(lnv[:], var[:].bitcast(F32), AF.Ln)
        rstd = spool.tile([1, S], F32R, name="rstd", tag="ln_rstd")
        nc.scalar.activation(rstd[:], lnv[:], AF.Exp, scale=-0.5)
        negmu_ps = psum_pool.tile([128, S], F32, name="negmu_ps", tag="bc0")
        nc.tensor.matmul(negmu_ps[:], neg_row[:], mu[:], start=True, stop=True)
        nc.vector.tensor_tensor(y[:, 0, :], y[:, 0, :].bitcast(F32),
                                negmu_ps[:], op=OP.add)
        for dt in range(NT):
            rg = rgp.tile([128, S], F32, name=f"rg{dt}", tag=f"rg{dt % 2}")
            nc.tensor.matmul(rg[:], g_rows[:, g_off + dt * 128:g_off + (dt + 1) * 128],
                             rstd[:], start=True, stop=True)
            nc.vector.tensor_tensor(out[:, dt, :], y[:, dt, :].bitcast(F32),
                                    rg[:], op=OP.mult)
            if bias_col is not None:
                nc.scalar.activation(out[:, dt, :], out[:, dt, :],
                                     AF.Identity, bias=bias_col[:, dt:dt + 1])
            if dt + 1 < NT:
                nc.vector.tensor_tensor(y[:, dt + 1, :], y[:, dt + 1, :].bitcast(F32),
                                        negmu_ps[:], op=OP.add)
                # dep-chained warm keeper: waits on the tile just produced,
                # so it executes mid-tail instead of all-at-once.
                wt_ = psum_pool.tile([1, S], F32, name="wk", tag="warm")
                nc.tensor.matmul(wt_[:], ones_colb[:], out[:, dt, :],
                                 start=True, stop=True)

    # =======================================================================
    # embedding: gather + add + transpose to feature-major + LN -> xT
    egr = pool.tile([1, D], F32R, name="egr")
    nc.sync.dma_start(egr[:], d["d_egr"].ap()[:, :].bitcast(F32R))
    with (
        tc.tile_pool(name="emb_sb", bufs=3) as embp,
        tc.tile_pool(name="emb_ps", bufs=3, space="PSUM") as embps,
        tc.tile_pool(name="emb_wm", bufs=1, space="PSUM") as embwm,
    ):
        for st in range(NST):
            idst = embp.tile([128, 1], I32, name="idst", tag="idst")
            nc.sync.dma_start(idst[:], d["d_ids"].ap()[st * 128:(st + 1) * 128, :])
            ttst = embp.tile([128, 1], I32, name="ttst", tag="ttst")
            nc.sync.dma_start(ttst[:], d["d_tti"].ap()[st * 128:(st + 1) * 128, :])
            x0 = embp.tile([128, D], F32, name="x0", tag="x0")
            nc.gpsimd.indirect_dma_start(
                out=x0[:], out_offset=None, in_=d["d_wemb"].ap(),
                in_offset=bass.IndirectOffsetOnAxis(ap=idst[:, :1], axis=0))
            tg = embp.tile([128, D], F32, name="tg", tag="tg")
            nc.gpsimd.indirect_dma_start(
                out=tg[:], out_offset=None, in_=d["d_temb"].ap(),
                in_offset=bass.IndirectOffsetOnAxis(ap=ttst[:, :1], axis=0))
            pg = embp.tile([128, D], F32, name="pg", tag="pg")
            nc.sync.dma_start(pg[:], d["d_pemb"].ap()[st * 128:(st + 1) * 128, :])
            nc.vector.tensor_tensor(x0[:], x0[:], tg[:], op=OP.add)
            nc.vector.tensor_tensor(x0[:], x0[:], pg[:], op=OP.add)
            for dt in range(NT):
                trp = embps.tile([128, 128], F32, name="trp", tag="trp")
                nc.tensor.transpose(trp[:], x0[:, dt * 128:(dt + 1) * 128], ident[:])
                nc.vector.tensor_copy(ybuf[:, dt, st * 128:(st + 1) * 128], trp[:])
            # warm keeper chained on this chunk's transposed output
            wt_ = embwm.tile([1, 128], F32, name="ewk", tag="warm")
            nc.tensor.matmul(wt_[:], ones_col[:],
                             ybuf[:, NT - 1, st * 128:(st + 1) * 128],
                             start=True, stop=True)
    with (
        tc.tile_pool(name="eln_ps", bufs=1, space="PSUM") as elnps,
        tc.tile_pool(name="eln_rg", bufs=2, space="PSUM") as elnrg,
    ):
        sqe = []
        for dt in range(NT):
            sqt = sqpool.tile([128, S], BF16, name=f"sqe{dt}", tag=f"sq2_{dt}")
            nc.vector.tensor_tensor(sqt[:], ybuf[:, dt, :].bitcast(F32),
                                    ybuf[:, dt, :].bitcast(F32), op=OP.mult)
            sqe.append(sqt)
        layernorm_fold(ybuf, sqe, egr, 0, 1e-12, xT, elnps, elnrg)

    # =======================================================================
    # transformer layers
    for l in range(L):
        # ---- per-layer params (two packed DMAs) --------------------------
        colsc = ppool.tile([128, 30], F32, name="colsc", tag="colsc")
        nc.sync.dma_start(colsc[:], d["d_cols"].ap()[l])
        rows = rpool.tile([1, 4 * D], F32R, name="rows", tag="rows")
        nc.sync.dma_start(rows[:], d["d_rows"].ap()[l].rearrange("(o e) -> o e", o=1).bitcast(F32R))
        bqc = colsc[:, 0:6]
        bkc = colsc[:, 6:12]
        beffc = colsc[:, 12:18]
        fgc = colsc[:, 18:24]
        fbc = colsc[:, 24:30]
        bvr = rows[:, 0:D]
        bor = rows[:, D:2 * D]
        wesum = rows[:, 2 * D:3 * D]

        # ---- attention-scope psum pools ----------------------------------
        with (
            tc.tile_pool(name="accp", bufs=2, space="PSUM") as accp,
            tc.tile_pool(name="awp", bufs=1, space="PSUM") as awp,
            tc.tile_pool(name="scp", bufs=1, space="PSUM") as scp,
            tc.tile_pool(name="ctxp", bufs=1, space="PSUM") as ctxp,
            tc.tile_pool(name="rcp", bufs=1, space="PSUM") as rcp,
        ):
            # ---- V (seq-major, two column halves) ------------------------
            wva = vwp.tile([128, NT, 512], BF16, name="wva", tag="va")
            nc.sync.dma_start(wva[:], d["d_wva"].ap()[l])
            wvb = vwp.tile([128, NT, 256], BF16, name="wvb", tag="vb")
            nc.sync.dma_start(wvb[:], d["d_wvb"].ap()[l])
            for st in range(NST):
                acc = accp.tile([128, 512], F32, name=f"va{st}", tag="acc")
                for dt in range(NT):
                    nc.tensor.matmul(acc[:], xT[:, dt, st * 128:(st + 1) * 128],
                                     wva[:, dt, :], start=(dt == 0), stop=False)
                nc.tensor.matmul(acc[:], one_row[:], bvr[0:1, 0:512],
                                 start=False, stop=True, skip_group_check=True)
                nc.vector.tensor_copy(
                    vaug[:, st, 0:8, 0:DK],
                    acc[:].rearrange("p (a b) -> p a b", a=8))
            for st in range(NST):
                acc = accp.tile([128, 256], F32, name=f"vb{st}", tag="acc")
                for dt in range(NT):
                    nc.tensor.matmul(acc[:], xT[:, dt, st * 128:(st + 1) * 128],
                                     wvb[:, dt, :], start=(dt == 0), stop=False)
                nc.tensor.matmul(acc[:], one_row[:], bvr[0:1, 512:768],
                                 start=False, stop=True, skip_group_check=True)
                nc.vector.tensor_copy(
                    vaug[:, st, 8:12, 0:DK],
                    acc[:].rearrange("p (a b) -> p a b", a=4))

            # ---- attention machinery -------------------------------------
            e_tiles = [None] * NP

            def emit_qk(p):
                for (wpool_, wd, bcol, dst, nm) in (
                        (wqp, d["d_wq"], bqc, qT, "q"), (wkp, d["d_wk"], bkc, kT, "k")):
                    wt = wpool_.tile([128, NT, 128], BF16, name=f"w{nm}{p}", tag=f"w{nm}")
                    nc.sync.dma_start(wt[:], wd.ap()[l, p])
                    acc = accp.tile([128, S], F32, name=f"{nm}{p}", tag="acc")
                    for dt in range(NT):
                        nc.tensor.matmul(acc[:], wt[:, dt, :], xT[:, dt, :],
                                         start=(dt == 0), stop=(dt == NT - 1))
                    nc.vector.tensor_scalar(dst[:, p, :], acc[:],
                                            bcol[:, p:p + 1], None, OP.add)

            def emit_scores(p):
                ets = []
                for kt in range(NST):
                    sc = scp.tile([128, 2, S], F32, name=f"sc{kt}", tag="sc")
                    for hh in range(2):
                        lo, hi = hh * 64, hh * 64 + 64
                        nc.tensor.matmul(
                            sc[:, hh, :], kT[lo:hi, p, kt * 128:(kt + 1) * 128],
                            qT[lo:hi, p, :], start=True, stop=True)
                    et = epool.tile([128, 2, S], BF16, name=f"e{kt}", tag="e")
                    nc.scalar.activation(et[:, :, :], sc[:, :, :], AF.Exp,
                                         bias=maskc[:, kt:kt + 1])
                    # warm keeper chained on the exp tile: the ACT-bound
                    # softmax window would otherwise idle the PE past the
                    # HAM MID window and re-throttle the clock to 1.2GHz.
                    wt_ = awp.tile([1, S], F32, name="awk", tag="awm")
                    nc.tensor.matmul(wt_[:], ones_colb[:], et[:, 0, :],
                                     start=True, stop=True)
                    ets.append(et)
                e_tiles[p] = ets

            def emit_pv(p):
                ets = e_tiles[p]
                cpss = []
                for hh in range(2):
                    h = 2 * p + hh
                    cps = ctxp.tile([DK + 1, S], F32, name=f"cps{hh}", tag=f"ctx{hh}")
                    for kt in range(NST):
                        nc.tensor.matmul(cps[:],
                                         vaug[:, kt, h, 0:DK + 1],
                                         ets[kt][:, hh, :],
                                         start=(kt == 0), stop=(kt == NST - 1))
                    cpss.append(cps)
                # reciprocal of the denominator row (partition 64)
                recips = []
                for hh in range(2):
                    nld = spool.tile([65, S], F32, name=f"nld{hh}", tag=f"nlden{hh}")
                    nc.scalar.activation(nld[64:65, :], cpss[hh][64:65, :], AF.Ln)
                    recip = spool.tile([65, S], F32R, name=f"rcp{hh}", tag=f"recip{hh}")
                    nc.scalar.activation(recip[64:65, :], nld[64:65, :], AF.Exp,
                                         scale=-1.0)
                    recips.append(recip)
                rps = rcp.tile([128, S], F32, name="rps", tag="rc")
                nc.tensor.matmul(rps[:], sel64[64:65, 0, :], recips[0][64:65, :],
                                 start=True, stop=False)
                nc.tensor.matmul(rps[:], sel64[64:65, 1, :], recips[1][64:65, :],
                                 start=False, stop=True, skip_group_check=True)
                rsb = spool.tile([128, S], F32, name="rsb", tag="rsb")
                nc.vector.tensor_copy(rsb[:], rps[:])
                for hh in range(2):
                    lo = hh * 64
                    nc.vector.tensor_tensor(cT[lo:lo + DK, p, :], cpss[hh][0:DK, :],
                                            rsb[lo:lo + DK, :], op=OP.mult)

            # pipeline: Q/K + scores run ahead; PV trails by 2 pairs
            for p in range(NP):
                emit_qk(p)
                if p >= 2:
                    emit_pv(p - 2)
                emit_scores(p)
            emit_pv(NP - 2)
            emit_pv(NP - 1)

            # ---- Wo + residual -> ybuf; squares chase for LN1 ------------
            sq1 = [None] * NT
            for et in range(NT):
                wt = wop.tile([128, NT, 128], BF16, name=f"wo{et}", tag="wo")
                nc.sync.dma_start(wt[:], d["d_wo"].ap()[l, et])
                acc = accp.tile([128, S], F32, name=f"o{et}", tag="acc")
                for dt in range(NT):
                    nc.tensor.matmul(acc[:], wt[:, dt, :], cT[:, dt, :],
                                     start=(dt == 0), stop=False)
                nc.tensor.matmul(acc[:], bor[0:1, et * 128:(et + 1) * 128],
                                 ones_s[:], start=False, stop=True,
                                 skip_group_check=True)
                nc.vector.tensor_tensor(ybuf[:, et, :], acc[:],
                                        xT[:, et, :], op=OP.add)
                sqt = sqpool.tile([128, S], BF16, name=f"sq1_{et}", tag=f"sq1_{et}")
                nc.vector.tensor_tensor(sqt[:], ybuf[:, et, :].bitcast(F32),
                                        ybuf[:, et, :].bitcast(F32), op=OP.mult)
                sq1[et] = sqt

        # ---- LN1 (folded into Weff) + FFN --------------------------------
        with (
            tc.tile_pool(name="lnp", bufs=1, space="PSUM") as lnp,
            tc.tile_pool(name="yaccp", bufs=3, space="PSUM") as yaccp,
        ):
            s0 = lnp.tile([1, S], F32, name="s0", tag="st0")
            s1t = lnp.tile([33, S], F32, name="s1t", tag="st1")
            for dt in range(NT):
                nc.tensor.matmul(s0[:], ones_col[:], ybuf[:, dt, :],
                                 start=(dt == 0), stop=(dt == NT - 1))
                nc.tensor.matmul(s1t[32:33, :], ones_colb[:], sq1[dt][:],
                                 start=(dt == 0), stop=(dt == NT - 1))
            negmu1 = spool.tile([1, S], F32R, name="negmu1", tag="negmu1")
            nc.vector.tensor_scalar(negmu1[:], s0[:], -1.0 / D, None, OP.mult)
            msq = spool.tile([1, S], F32, name="msq1", tag="ln_msq")
            nc.vector.tensor_scalar(msq[:], s1t[32:33, :], 1.0 / D, 1e-5, OP.mult, OP.add)
            musq = spool.tile([1, S], F32, name="musq1", tag="ln_musq")
            nc.vector.tensor_tensor(musq[:], negmu1[:].bitcast(F32),
                                    negmu1[:].bitcast(F32), op=OP.mult)
            var = spool.tile([1, S], F32, name="var1", tag="ln_var")
            nc.vector.tensor_tensor(var[:], msq[:], musq[:], op=OP.subtract)
            lnv = spool.tile([1, S], F32, name="lnv1", tag="ln_lnv")
            nc.scalar.activation(lnv[:], var[:], AF.Ln)
            rstd1 = spool.tile([1, S], F32R, name="rstd1", tag="ln_rstd")
            nc.scalar.activation(rstd1[:], lnv[:], AF.Exp, scale=-0.5)

            # FFN: y2 = rstd1 .col* (Weffg @ ybuf - mu1 x wesum) + beff
            rstd1b = spool.tile([128, S], F32, name="rstd1b", tag="rstd1b")
            sq2 = [None] * NT

            def emit_ffn_group(et):
                wt = wfp.tile([128, NT, 128], F32R, name=f"wf{et}", tag="wf")
                nc.sync.dma_start(wt[:], d["d_wf"].ap()[l, et].bitcast(F32R))
                acc = yaccp.tile([128, S], F32, name=f"y{et}", tag="yacc")
                for dt in range(NT):
                    nc.tensor.matmul(acc[:], wt[:, dt, :], ybuf[:, dt, :],
                                     start=(dt == 0), stop=False)
                nc.tensor.matmul(acc[:], wesum[0:1, et * 128:(et + 1) * 128],
                                 negmu1[:], start=False, stop=True,
                                 skip_group_check=True)
                return acc

            def emit_ffn_evac(et, acc):
                nc.vector.tensor_tensor(ybuf2[:, et, :], acc[:], rstd1b[:],
                                        op=OP.mult)
                nc.scalar.activation(ybuf2[:, et, :], ybuf2[:, et, :],
                                     AF.Identity, bias=beffc[:, et:et + 1])
                sqt = sqpool.tile([128, S], BF16, name=f"sq2_{et}", tag=f"sq2_{et}")
                nc.vector.tensor_tensor(sqt[:], ybuf2[:, et, :].bitcast(F32),
                                        ybuf2[:, et, :].bitcast(F32), op=OP.mult)
                sq2[et] = sqt

            accs = [None] * NT
            accs[0] = emit_ffn_group(0)
            accs[1] = emit_ffn_group(1)
            # rstd broadcast after two groups: the ACT chain has finished by
            # then, so the PE never stalls; evacs unblock before group 3
            # needs a psum slot.
            rstd1_ps = lnp.tile([128, S], F32, name="rstd1_ps", tag="bc1")
            nc.tensor.matmul(rstd1_ps[:], one_row[:], rstd1[:], start=True, stop=True)
            nc.vector.tensor_copy(rstd1b[:], rstd1_ps[:])
            emit_ffn_evac(0, accs[0])
            for et in range(2, NT):
                accs[et] = emit_ffn_group(et)
                emit_ffn_evac(et - 1, accs[et - 1])
            emit_ffn_evac(NT - 1, accs[NT - 1])

        # ---- LN2 -> xT (next layer input); beta folded downstream except
        # for the final layer (the kernel output needs the full affine).
        with (
            tc.tile_pool(name="ln2p", bufs=1, space="PSUM") as ln2p,
            tc.tile_pool(name="ln2rg", bufs=2, space="PSUM") as ln2rg,
        ):
            layernorm_fold(ybuf2, sq2, rows, 3 * D, 1e-5, xT, ln2p, ln2rg,
                           bias_col=fbc if l == L - 1 else None)

    # =======================================================================
    # output: transpose xT -> [S, D] and DMA out
    with (
        tc.tile_pool(name="out_sb", bufs=2) as outp,
        tc.tile_pool(name="out_ps", bufs=2, space="PSUM") as outps,
    ):
        for st in range(NST):
            ops_t = outps.tile([128, D], BF16, name="ops", tag="ops")
            for dt in range(NT):
                nc.tensor.transpose(ops_t[:, dt * 128:(dt + 1) * 128],
                                    xT[:, dt, st * 128:(st + 1) * 128],
                                    ident16[:])
            osb = outp.tile([128, D], F32, name="osb", tag="osb")
            nc.vector.tensor_copy(osb[:], ops_t[:])
            nc.sync.dma_start(d["d_out"].ap()[st * 128:(st + 1) * 128, :], osb[:])


# ---------------------------------------------------------------------------
def _pack_dd(w):
    """[L, Din, Dout] -> [L, et, p, n, e] with w[l, n*128+p, et*128+e]."""
    Lw = w.shape[0]
    return np.ascontiguousarray(
        w.reshape(Lw, NT, 128, NT, 128).transpose(0, 3, 2, 1, 4))


def kernel(**inputs):
    global _NC_CACHE
    if _NC_CACHE is None:
        _NC_CACHE = _build_nc()
    nc = _NC_CACHE

    import ml_dtypes
    f32 = lambda a: np.ascontiguousarray(np.asarray(a), dtype=np.float32)
    bf = lambda a: np.ascontiguousarray(a.astype(ml_dtypes.bfloat16))

    Wq = f32(inputs["Wq"]) * SCALE
    bq = f32(inputs["bq"]) * SCALE
    Wk = f32(inputs["Wk"])
    Wv = f32(inputs["Wv"])
    Wo = f32(inputs["Wo"])
    W1 = f32(inputs["W1"])
    W2 = f32(inputs["W2"])
    b1 = f32(inputs["b1"])
    b2 = f32(inputs["b2"])
    ag = f32(inputs["attn_ln_g"])
    ab = f32(inputs["attn_ln_b"])
    fg_ln = f32(inputs["ffn_ln_g"])
    fb_ln = f32(inputs["ffn_ln_b"])
    eb_ln = f32(inputs["emb_ln_b"])

    # collapse the (linear) FFN: y2 = W2 @ (W1 @ a + b1) + b2 with
    # a = g * z + b  =>  y2 = Weffg @ z + beff
    Weff = np.einsum("ldf,lfe->lde", W2, W1)          # [L, D, D]
    Weffg = Weff * ag[:, None, :]                      # fold LN1 gamma
    beff = (np.einsum("lde,le->ld", Weff, ab)
            + np.einsum("ldf,lf->ld", W2, b1) + b2)    # fold LN1 beta + biases
    wesum = Weffg.sum(axis=2)                          # [L, D]

    # fold the preceding LN's beta (emb LN for layer 0, LN2 of layer l-1
    # otherwise) into the QKV/residual biases: x_l = xT' + b_prev.
    b_prev = np.concatenate([eb_ln[None], fb_ln[:-1]], axis=0)   # [L, D]
    bq = bq + np.einsum("led,ld->le", Wq, b_prev)
    bk = f32(inputs["bk"]) + np.einsum("led,ld->le", Wk, b_prev)
    bv = f32(inputs["bv"]) + np.einsum("led,ld->le", Wv, b_prev)
    bo = f32(inputs["bo"]) + b_prev

    WqT = Wq.transpose(0, 2, 1)
    WkT = Wk.transpose(0, 2, 1)
    WvT = Wv.transpose(0, 2, 1)
    WoT = Wo.transpose(0, 2, 1)
    WfT = Weffg.transpose(0, 2, 1)

    WvA = WvT.reshape(L, NT, 128, D)                   # [l, n, p, e]
    wva = np.ascontiguousarray(WvA[:, :, :, 0:512].transpose(0, 2, 1, 3))
    wvb = np.ascontiguousarray(WvA[:, :, :, 512:768].transpose(0, 2, 1, 3))

    r = lambda a: a.reshape(L, NT, 128).transpose(0, 2, 1)  # [L,128,NT] cols
    cols = np.concatenate([r(bq), r(bk), r(beff), r(fg_ln), r(fb_ln)],
                          axis=2).astype(np.float32)   # [L, 128, 30]
    rows = np.concatenate([bv, bo, wesum, fg_ln],
                          axis=1).astype(np.float32)   # [L, 4*D]

    shared = {
        "wemb": f32(inputs["word_emb"]),
        "pemb": f32(inputs["pos_emb"])[:S],
        "temb": f32(inputs["type_emb"]),
        "egr": f32(inputs["emb_ln_g"]).reshape(1, D),
        "wq": bf(_pack_dd(WqT)),
        "wk": bf(_pack_dd(WkT)),
        "wo": bf(_pack_dd(WoT)),
        "wf": _pack_dd(WfT).astype(np.float32),
        "wva": bf(wva), "wvb": bf(wvb),
        "cols": np.ascontiguousarray(cols),
        "rows": np.ascontiguousarray(rows),
        "ones128": np.ones(128, np.float32),
        "ones128b": np.ones(128, ml_dtypes.bfloat16),
        "ones512": np.ones((1, 512), np.float32),
        "neg1": np.full((1, 128), -1.0, np.float32),
        "selA": np.concatenate([np.ones((1, 64)), np.zeros((1, 64))], 1).astype(np.float32),
        "selB": np.concatenate([np.zeros((1, 64)), np.ones((1, 64))], 1).astype(np.float32),
        "onesgridb": np.ones((128, NST * H), ml_dtypes.bfloat16),
    }
    ids = np.asarray(inputs["input_ids"]).astype(np.int32)
    tti = np.asarray(inputs["token_type_ids"]).astype(np.int32)
    am = np.asarray(inputs["attention_mask"]).astype(np.float32)
    in_maps = []
    for c in range(B):
        in_maps.append({
            **shared,
            "ids": ids[c].reshape(S, 1),
            "tti": tti[c].reshape(S, 1),
            "maskadd": np.where(am[c] == 0, -1e9, 0.0).astype(np.float32),
        })
    res = bass_utils.run_bass_kernel_spmd(
        nc, in_maps, core_ids=list(range(B)), trace=False)
    out = np.stack([res.results[c]["out"] for c in range(B)], axis=0)
    return out.astype(np.float32)


# revision 19
# speedup vs baseline: 1.2748x; 1.2748x over previous
"""BERT-base (12-layer, B=8, S=512, D=768, H=12, F=3072) forward pass on 8
Trainium2 NeuronCores — v2.

Strategy: data-parallel over batch (1 sequence per core, no collectives).
Key structure (per core, activations feature-major xT[D, S] in SBUF):
  - the FFN in this model is LINEAR (no activation between W1/W2), so
    W2@W1 collapses on the host into one 768x768 matrix Weff; the attn-LN
    affine (g,b) folds into Weff/beff as well, and the LN normalize
    (mean/rstd) folds into the GEMM via a rank-1 correction + per-column
    scale at PSUM evacuation. The FFN costs 1/4 of the naive FLOPs and
    layer-norm #1 never stalls the PE.
  - softmax denominators fall out of a ones-augmented V column (PV matmul
    M=65); reciprocals via Ln/Exp on the denominator row (partition 64).
  - weights are host-repacked so every weight DMA is contiguous per
    partition (1.5-6KB lines instead of 256B).
  - emission interleaves Q/K projections with scores+exp per head-pair so
    the ACT-bound softmax overlaps PE GEMM work; dummy warm matmuls keep
    the PE HAM clock at 2.4GHz across LN chains.
"""
import numpy as np

import concourse.bass as bass
import concourse.mybir as mybir
import concourse.tile as tile
from concourse import bass_utils
from concourse.masks import make_identity

AF = mybir.ActivationFunctionType
OP = mybir.AluOpType
F32 = mybir.dt.float32
F32R = mybir.dt.float32r
BF16 = mybir.dt.bfloat16
I32 = mybir.dt.int32

B, S, D, H, F, L, V = 8, 512, 768, 12, 3072, 12, 30522
DK = D // H
SCALE = 1.0 / float(np.sqrt(DK))
NT = D // 128      # 6 feature tiles
NST = S // 128     # 4 sequence tiles
NP = H // 2        # 6 head pairs

_NC_CACHE = None


# ---------------------------------------------------------------------------
# wait-slot legalization: walrus codegen allows only ONE sync-wait command on
# TPB instructions; hoist excess waits into standalone EventSemaphores.
def _legalize_waits(nc):
    skip = (mybir.InstEventSemaphore, mybir.InstNoOp)
    n = 0
    for fn in nc.m.functions:
        for blk in fn.blocks:
            out = []
            for inst in blk.instructions:
                si = inst.sync_info
                if si is not None and si.on_wait and not isinstance(inst, skip) \
                        and len(si.on_wait) > 1:
                    waits = list(si.on_wait)
                    for j, w in enumerate(waits[:-1]):
                        ev = mybir.InstEventSemaphore(
                            name=f"{inst.name}-lgw{j}", ins=[], outs=[],
                            sync_info=mybir.SyncInfo(on_wait=[w], on_update=[]),
                        )
                        ev.engine = inst.engine
                        out.append(ev)
                        n += 1
                    inst.sync_info = mybir.SyncInfo(
                        on_wait=[waits[-1]], on_update=list(si.on_update))
                out.append(inst)
            try:
                blk.instructions = out
            except Exception:
                blk.instructions.clear()
                blk.instructions.extend(out)
    return n


def _build_nc():
    nc = bass.Bass("TRN2", target_bir_lowering=False, debug=False,
                   enable_asserts=False, num_devices=8)

    # ---- DRAM I/O ---------------------------------------------------------
    d = {}
    d["d_ids"] = nc.dram_tensor("ids", [S, 1], I32, kind="ExternalInput")
    d["d_tti"] = nc.dram_tensor("tti", [S, 1], I32, kind="ExternalInput")
    d["d_mask"] = nc.dram_tensor("maskadd", [S], F32, kind="ExternalInput")
    d["d_wemb"] = nc.dram_tensor("wemb", [V, D], F32, kind="ExternalInput")
    d["d_pemb"] = nc.dram_tensor("pemb", [S, D], F32, kind="ExternalInput")
    d["d_temb"] = nc.dram_tensor("temb", [2, D], F32, kind="ExternalInput")

    # repacked weights: [L, et, p, n, e] so each (l, et) chunk is contiguous
    d["d_wq"] = nc.dram_tensor("wq", [L, NT, 128, NT, 128], BF16, kind="ExternalInput")
    d["d_wk"] = nc.dram_tensor("wk", [L, NT, 128, NT, 128], BF16, kind="ExternalInput")
    d["d_wo"] = nc.dram_tensor("wo", [L, NT, 128, NT, 128], BF16, kind="ExternalInput")
    d["d_wf"] = nc.dram_tensor("wf", [L, NT, 128, NT, 128], F32, kind="ExternalInput")
    d["d_wva"] = nc.dram_tensor("wva", [L, 128, NT, 512], BF16, kind="ExternalInput")
    d["d_wvb"] = nc.dram_tensor("wvb", [L, 128, NT, 256], BF16, kind="ExternalInput")
    # packed per-layer params: columns [128, 30] and rows [1, 3*768]
    d["d_cols"] = nc.dram_tensor("cols", [L, 128, 30], F32, kind="ExternalInput")
    d["d_rows"] = nc.dram_tensor("rows", [L, 4 * D], F32, kind="ExternalInput")
    d["d_egr"] = nc.dram_tensor("egr", [1, D], F32, kind="ExternalInput")
    d["d_ones"] = nc.dram_tensor("ones128", [128], F32, kind="ExternalInput")
    d["d_onesb"] = nc.dram_tensor("ones128b", [128], BF16, kind="ExternalInput")
    d["d_ones512"] = nc.dram_tensor("ones512", [1, 512], F32, kind="ExternalInput")
    d["d_neg1"] = nc.dram_tensor("neg1", [1, 128], F32, kind="ExternalInput")
    d["d_selA"] = nc.dram_tensor("selA", [1, 128], F32, kind="ExternalInput")
    d["d_selB"] = nc.dram_tensor("selB", [1, 128], F32, kind="ExternalInput")
    d["d_onesgb"] = nc.dram_tensor("onesgridb", [128, NST * H], BF16, kind="ExternalInput")
    d["d_out"] = nc.dram_tensor("out", [S, D], F32, kind="ExternalOutput")

    with tile.TileContext(nc) as tc:
        _emit(nc, tc, d)
    _legalize_waits(nc)
    return nc


def _emit(nc, tc, d):
    import contextlib
    ctx = contextlib.ExitStack()
    with ctx:
        _emit_body(nc, tc, d, ctx)


def _emit_body(nc, tc, d, ctx):
    pool = ctx.enter_context(tc.tile_pool(name="persist", bufs=1))
    wqp = ctx.enter_context(tc.tile_pool(name="wqp", bufs=2))
    wkp = ctx.enter_context(tc.tile_pool(name="wkp", bufs=2))
    wop = ctx.enter_context(tc.tile_pool(name="wop", bufs=4))
    wfp = ctx.enter_context(tc.tile_pool(name="wfp", bufs=4))
    vwp = ctx.enter_context(tc.tile_pool(name="vwp", bufs=2))
    ppool = ctx.enter_context(tc.tile_pool(name="params", bufs=2))
    rpool = ctx.enter_context(tc.tile_pool(name="rowsp", bufs=1))
    epool = ctx.enter_context(tc.tile_pool(name="epool", bufs=10))
    spool = ctx.enter_context(tc.tile_pool(name="smalls", bufs=1))
    sqpool = ctx.enter_context(tc.tile_pool(name="sqp", bufs=1))

    # ---- persistent constants --------------------------------------------
    ones_col = pool.tile([128, 1], F32R, name="ones_col")
    nc.sync.dma_start(ones_col[:], d["d_ones"].ap().rearrange("(p o) -> p o", o=1).bitcast(F32R))
    ones_colb = pool.tile([128, 1], BF16, name="ones_colb")
    nc.sync.dma_start(ones_colb[:], d["d_onesb"].ap().rearrange("(p o) -> p o", o=1))
    one_row = pool.tile([1, 128], F32R, name="one_row")
    nc.sync.dma_start(one_row[:], d["d_ones"].ap().rearrange("(o p) -> o p", o=1).bitcast(F32R))
    ones_s = pool.tile([1, 512], F32R, name="ones_s")
    nc.sync.dma_start(ones_s[:], d["d_ones512"].ap()[:, :].bitcast(F32R))
    neg_row = pool.tile([1, 128], F32R, name="neg_row")
    nc.sync.dma_start(neg_row[:], d["d_neg1"].ap()[:, :].bitcast(F32R))
    # head-select rows living on partition 64 (same partition as the PV
    # denominator row) so the broadcast matmul's operands share a base.
    sel64 = pool.tile([65, 2, 128], F32R, name="sel64")
    nc.sync.dma_start(sel64[64:65, 0, :], d["d_selA"].ap()[:, :].bitcast(F32R))
    nc.sync.dma_start(sel64[64:65, 1, :], d["d_selB"].ap()[:, :].bitcast(F32R))
    ident = pool.tile([128, 128], F32, name="ident")
    make_identity(nc, ident[:])
    ident16 = pool.tile([128, 128], BF16, name="ident16")
    make_identity(nc, ident16[:])
    maskc = pool.tile([128, NST], F32, name="maskc")
    nc.sync.dma_start(maskc[:], d["d_mask"].ap().rearrange("(n p) -> p n", p=128))

    # ---- persistent activations ------------------------------------------
    xT = pool.tile([128, NT, S], BF16, name="xT")       # layer input, feature-major
    qT = pool.tile([128, NT, S], BF16, name="qT")
    kT = pool.tile([128, NT, S], BF16, name="kT")
    cT = pool.tile([128, NT, S], BF16, name="cT")       # ctx, feature-major
    ybuf = pool.tile([128, NT, S], F32R, name="ybuf")   # post-Wo residual
    ybuf2 = pool.tile([128, NT, S], F32R, name="ybuf2")  # post-FFN (pre-LN2)
    vaug = pool.tile([128, NST, H, DK + 1], BF16, name="vaug")
    nc.sync.dma_start(
        vaug[:, :, :, DK:DK + 1],
        d["d_onesgb"].ap().rearrange("p (a b) -> p a b", a=NST)[:, :, :],
    )

    def warm_mm(wps):
        t = wps.tile([128, S], F32, name="warm", tag="warm")
        nc.tensor.matmul(t[:], one_row[:], ones_s[:], start=True, stop=True)

    # =======================================================================
    # folded layernorm (embedding LN + LN2): y [128, nt, S] F32R ->
    # out = (y - mu) * rstd * g  (gamma via rank-1 g x rstd broadcasts; beta
    # is host-folded into downstream biases, except `bias_col` for the last
    # layer). sq_t are precomputed squares of y. Dummy warm matmuls chained
    # on each output tile keep the PE HAM clock warm across the DVE tail.
    def layernorm_fold(y, sq_t, g_rows, g_off, eps, out, psum_pool, rgp,
                       bias_col=None):
        s0 = psum_pool.tile([1, S], F32, name="s0", tag="st0")
        s1t = psum_pool.tile([33, S], F32, name="s1t", tag="st1")
        # col-packed stats: s0 -> col group 0, s1 -> col group 1 (concurrent)
        for dt in range(NT):
            nc.tensor.matmul(s0[:], ones_col[:], y[:, dt, :],
                             start=(dt == 0), stop=(dt == NT - 1))
            nc.tensor.matmul(s1t[32:33, :], ones_colb[:], sq_t[dt][:],
                             start=(dt == 0), stop=(dt == NT - 1))
        mu = spool.tile([1, S], F32R, name="mu", tag="ln_mu")
        nc.vector.tensor_scalar(mu[:], s0[:], 1.0 / D, None, OP.mult)
        msq = spool.tile([1, S], F32, name="msq", tag="ln_msq")
        nc.vector.tensor_scalar(msq[:], s1t[32:33, :], 1.0 / D, eps, OP.mult, OP.add)
        musq = spool.tile([1, S], F32, name="musq", tag="ln_musq")
        nc.vector.tensor_tensor(musq[:], mu[:].bitcast(F32), mu[:].bitcast(F32), op=OP.mult)
        var = spool.tile([1, S], F32R, name="var", tag="ln_var")
        nc.vector.tensor_tensor(var[:], msq[:], musq[:], op=OP.subtract)
        # warm keeper chained on var: bridges the stats->rstd ACT stretch
        wv_ = psum_pool.tile([128, S], F32, name="wv", tag="warm")
        nc.tensor.matmul(wv_[:], one_row[:], var[:], start=True, stop=True)
        lnv = spool.tile([1, S], F32, name="lnv", tag="ln_lnv")
        nc.scalar.activation(lnv[:], var[:].bitcast(F32), AF.Ln)
        rstd = spool.tile([1, S], F32R, name="rstd", tag="ln_rstd")
        nc.scalar.activation(rstd[:], lnv[:], AF.Exp, scale=-0.5)
        negmu_ps = psum_pool.tile([128, S], F32, name="negmu_ps", tag="bc0")
        nc.tensor.matmul(negmu_ps[:], neg_row[:], mu[:], start=True, stop=True)
        nc.vector.tensor_tensor(y[:, 0, :], y[:, 0, :].bitcast(F32),
                                negmu_ps[:], op=OP.add)
        for dt in range(NT):
            rg = rgp.tile([128, S], F32, name=f"rg{dt}", tag=f"rg{dt % 2}")
            nc.tensor.matmul(rg[:], g_rows[:, g_off + dt * 128:g_off + (dt + 1) * 128],
                             rstd[:], start=True, stop=True)
            nc.vector.tensor_tensor(out[:, dt, :], y[:, dt, :].bitcast(F32),
                                    rg[:], op=OP.mult)
            if bias_col is not None:
                nc.scalar.activation(out[:, dt, :], out[:, dt, :],
                                     AF.Identity, bias=bias_col[:, dt:dt + 1])
            if dt + 1 < NT:
                nc.vector.tensor_tensor(y[:, dt + 1, :], y[:, dt + 1, :].bitcast(F32),
                                        negmu_ps[:], op=OP.add)
                # dep-chained warm keeper: waits on the tile just produced,
                # so it executes mid-tail instead of all-at-once.
                wt_ = psum_pool.tile([1, S], F32, name="wk", tag="warm")
                nc.tensor.matmul(wt_[:], ones_colb[:], out[:, dt, :],
                                 start=True, stop=True)

    # =======================================================================
    # embedding: gather + add + transpose to feature-major + LN -> xT
    egr = pool.tile([1, D], F32R, name="egr")
    nc.sync.dma_start(egr[:], d["d_egr"].ap()[:, :].bitcast(F32R))
    with (
        tc.tile_pool(name="emb_sb", bufs=3) as embp,
        tc.tile_pool(name="emb_ps", bufs=3, space="PSUM") as embps,
        tc.tile_pool(name="emb_wm", bufs=1, space="PSUM") as embwm,
    ):
        for st in range(NST):
            idst = embp.tile([128, 1], I32, name="idst", tag="idst")
            nc.sync.dma_start(idst[:], d["d_ids"].ap()[st * 128:(st + 1) * 128, :])
            ttst = embp.tile([128, 1], I32, name="ttst", tag="ttst")
            nc.sync.dma_start(ttst[:], d["d_tti"].ap()[st * 128:(st + 1) * 128, :])
            x0 = embp.tile([128, D], F32, name="x0", tag="x0")
            nc.gpsimd.indirect_dma_start(
                out=x0[:], out_offset=None, in_=d["d_wemb"].ap(),
                in_offset=bass.IndirectOffsetOnAxis(ap=idst[:, :1], axis=0))
            tg = embp.tile([128, D], F32, name="tg", tag="tg")
            nc.gpsimd.indirect_dma_start(
                out=tg[:], out_offset=None, in_=d["d_temb"].ap(),
                in_offset=bass.IndirectOffsetOnAxis(ap=ttst[:, :1], axis=0))
            pg = embp.tile([128, D], F32, name="pg", tag="pg")
            nc.sync.dma_start(pg[:], d["d_pemb"].ap()[st * 128:(st + 1) * 128, :])
            nc.vector.tensor_tensor(x0[:], x0[:], tg[:], op=OP.add)
            nc.vector.tensor_tensor(x0[:], x0[:], pg[:], op=OP.add)
            for dt in range(NT):
                trp = embps.tile([128, 128], F32, name="trp", tag="trp")
                nc.tensor.transpose(trp[:], x0[:, dt * 128:(dt + 1) * 128], ident[:])
                nc.vector.tensor_copy(ybuf[:, dt, st * 128:(st + 1) * 128], trp[:])
            # warm keeper chained on this chunk's transposed output
            wt_ = embwm.tile([1, 128], F32, name="ewk", tag="warm")
            nc.tensor.matmul(wt_[:], ones_col[:],
                             ybuf[:, NT - 1, st * 128:(st + 1) * 128],
                             start=True, stop=True)
    with (
        tc.tile_pool(name="eln_ps", bufs=1, space="PSUM") as elnps,
        tc.tile_pool(name="eln_rg", bufs=2, space="PSUM") as elnrg,
    ):
        sqe = []
        for dt in range(NT):
            sqt = sqpool.tile([128, S], BF16, name=f"sqe{dt}", tag=f"sq2_{dt}")
            nc.vector.tensor_tensor(sqt[:], ybuf[:, dt, :].bitcast(F32),
                                    ybuf[:, dt, :].bitcast(F32), op=OP.mult)
            sqe.append(sqt)
        layernorm_fold(ybuf, sqe, egr, 0, 1e-12, xT, elnps, elnrg)

    # =======================================================================
    # transformer layers
    for l in range(L):
        # ---- per-layer params (two packed DMAs) --------------------------
        colsc = ppool.tile([128, 30], F32, name="colsc", tag="colsc")
        nc.sync.dma_start(colsc[:], d["d_cols"].ap()[l])
        rows = rpool.tile([1, 4 * D], F32R, name="rows", tag="rows")
        nc.sync.dma_start(rows[:], d["d_rows"].ap()[l].rearrange("(o e) -> o e", o=1).bitcast(F32R))
        bqc = colsc[:, 0:6]
        bkc = colsc[:, 6:12]
        beffc = colsc[:, 12:18]
        fgc = colsc[:, 18:24]
        fbc = colsc[:, 24:30]
        bvr = rows[:, 0:D]
        bor = rows[:, D:2 * D]
        wesum = rows[:, 2 * D:3 * D]

        # ---- attention-scope psum pools ----------------------------------
        with (
            tc.tile_pool(name="accp", bufs=3, space="PSUM") as accp,
            tc.tile_pool(name="scp", bufs=1, space="PSUM") as scp,
            tc.tile_pool(name="ctxp", bufs=1, space="PSUM") as ctxp,
            tc.tile_pool(name="rcp", bufs=1, space="PSUM") as rcp,
        ):
            # ---- V (seq-major, two column halves) ------------------------
            wva = vwp.tile([128, NT, 512], BF16, name="wva", tag="va")
            nc.sync.dma_start(wva[:], d["d_wva"].ap()[l])
            wvb = vwp.tile([128, NT, 256], BF16, name="wvb", tag="vb")
            nc.sync.dma_start(wvb[:], d["d_wvb"].ap()[l])
            for st in range(NST):
                acc = accp.tile([128, 512], F32, name=f"va{st}", tag="acc")
                for dt in range(NT):
                    nc.tensor.matmul(acc[:], xT[:, dt, st * 128:(st + 1) * 128],
                                     wva[:, dt, :], start=(dt == 0), stop=False)
                nc.tensor.matmul(acc[:], one_row[:], bvr[0:1, 0:512],
                                 start=False, stop=True, skip_group_check=True)
                nc.vector.tensor_copy(
                    vaug[:, st, 0:8, 0:DK],
                    acc[:].rearrange("p (a b) -> p a b", a=8))
            for st in range(NST):
                acc = accp.tile([128, 256], F32, name=f"vb{st}", tag="acc")
                for dt in range(NT):
                    nc.tensor.matmul(acc[:], xT[:, dt, st * 128:(st + 1) * 128],
                                     wvb[:, dt, :], start=(dt == 0), stop=False)
                nc.tensor.matmul(acc[:], one_row[:], bvr[0:1, 512:768],
                                 start=False, stop=True, skip_group_check=True)
                nc.vector.tensor_copy(
                    vaug[:, st, 8:12, 0:DK],
                    acc[:].rearrange("p (a b) -> p a b", a=4))

            # ---- attention machinery -------------------------------------
            e_tiles = [None] * NP

            def emit_qk(p):
                for (wpool_, wd, bcol, dst, nm) in (
                        (wqp, d["d_wq"], bqc, qT, "q"), (wkp, d["d_wk"], bkc, kT, "k")):
                    wt = wpool_.tile([128, NT, 128], BF16, name=f"w{nm}{p}", tag=f"w{nm}")
                    nc.sync.dma_start(wt[:], wd.ap()[l, p])
                    acc = accp.tile([128, S], F32, name=f"{nm}{p}", tag="acc")
                    for dt in range(NT):
                        nc.tensor.matmul(acc[:], wt[:, dt, :], xT[:, dt, :],
                                         start=(dt == 0), stop=(dt == NT - 1))
                    nc.vector.tensor_scalar(dst[:, p, :], acc[:],
                                            bcol[:, p:p + 1], None, OP.add)

            def emit_scores(p):
                ets = []
                for kt in range(NST):
                    sc = scp.tile([128, 2, S], F32, name=f"sc{kt}", tag="sc")
                    for hh in range(2):
                        lo, hi = hh * 64, hh * 64 + 64
                        nc.tensor.matmul(
                            sc[:, hh, :], kT[lo:hi, p, kt * 128:(kt + 1) * 128],
                            qT[lo:hi, p, :], start=True, stop=True)
                    et = epool.tile([128, 2, S], BF16, name=f"e{kt}", tag="e")
                    nc.scalar.activation(et[:, :, :], sc[:, :, :], AF.Exp,
                                         bias=maskc[:, kt:kt + 1])
                    ets.append(et)
                e_tiles[p] = ets

            def emit_pv(p):
                ets = e_tiles[p]
                cpss = []
                for hh in range(2):
                    h = 2 * p + hh
                    cps = ctxp.tile([DK + 1, S], F32, name=f"cps{hh}", tag=f"ctx{hh}")
                    for kt in range(NST):
                        nc.tensor.matmul(cps[:],
                                         vaug[:, kt, h, 0:DK + 1],
                                         ets[kt][:, hh, :],
                                         start=(kt == 0), stop=(kt == NST - 1))
                    cpss.append(cps)
                # reciprocal of the denominator row (partition 64)
                recips = []
                for hh in range(2):
                    nld = spool.tile([65, S], F32, name=f"nld{hh}", tag=f"nlden{hh}")
                    nc.scalar.activation(nld[64:65, :], cpss[hh][64:65, :], AF.Ln)
                    recip = spool.tile([65, S], F32R, name=f"rcp{hh}", tag=f"recip{hh}")
                    nc.scalar.activation(recip[64:65, :], nld[64:65, :], AF.Exp,
                                         scale=-1.0)
                    recips.append(recip)
                rps = rcp.tile([128, S], F32, name="rps", tag="rc")
                nc.tensor.matmul(rps[:], sel64[64:65, 0, :], recips[0][64:65, :],
                                 start=True, stop=False)
                nc.tensor.matmul(rps[:], sel64[64:65, 1, :], recips[1][64:65, :],
                                 start=False, stop=True, skip_group_check=True)
                rsb = spool.tile([128, S], F32, name="rsb", tag="rsb")
                nc.vector.tensor_copy(rsb[:], rps[:])
                for hh in range(2):
                    lo = hh * 64
                    nc.vector.tensor_tensor(cT[lo:lo + DK, p, :], cpss[hh][0:DK, :],
                                            rsb[lo:lo + DK, :], op=OP.mult)

            # pipeline: Q/K + scores run ahead; PV trails by 2 pairs
            for p in range(NP):
                emit_qk(p)
                if p >= 2:
                    emit_pv(p - 2)
                emit_scores(p)
            emit_pv(NP - 2)
            emit_pv(NP - 1)

            # ---- Wo + residual -> ybuf; squares chase for LN1 ------------
            sq1 = [None] * NT
            for et in range(NT):
                wt = wop.tile([128, NT, 128], BF16, name=f"wo{et}", tag="wo")
                nc.sync.dma_start(wt[:], d["d_wo"].ap()[l, et])
                acc = accp.tile([128, S], F32, name=f"o{et}", tag="acc")
                for dt in range(NT):
                    nc.tensor.matmul(acc[:], wt[:, dt, :], cT[:, dt, :],
                                     start=(dt == 0), stop=False)
                nc.tensor.matmul(acc[:], bor[0:1, et * 128:(et + 1) * 128],
                                 ones_s[:], start=False, stop=True,
                                 skip_group_check=True)
                nc.vector.tensor_tensor(ybuf[:, et, :], acc[:],
                                        xT[:, et, :], op=OP.add)
                sqt = sqpool.tile([128, S], BF16, name=f"sq1_{et}", tag=f"sq1_{et}")
                nc.vector.tensor_tensor(sqt[:], ybuf[:, et, :].bitcast(F32),
                                        ybuf[:, et, :].bitcast(F32), op=OP.mult)
                sq1[et] = sqt

        # ---- LN1 (folded into Weff) + FFN --------------------------------
        with (
            tc.tile_pool(name="lnp", bufs=1, space="PSUM") as lnp,
            tc.tile_pool(name="yaccp", bufs=3, space="PSUM") as yaccp,
        ):
            s0 = lnp.tile([1, S], F32, name="s0", tag="st0")
            s1t = lnp.tile([33, S], F32, name="s1t", tag="st1")
            for dt in range(NT):
                nc.tensor.matmul(s0[:], ones_col[:], ybuf[:, dt, :],
                                 start=(dt == 0), stop=(dt == NT - 1))
            negmu1 = spool.tile([1, S], F32R, name="negmu1", tag="negmu1")
            nc.vector.tensor_scalar(negmu1[:], s0[:], -1.0 / D, None, OP.mult)

            # FFN: y2 = rstd1 .col* (Weffg @ ybuf - mu1 x wesum) + beff
            rstd1b = spool.tile([128, S], F32, name="rstd1b", tag="rstd1b")
            sq2 = [None] * NT

            def emit_ffn_group(et):
                wt = wfp.tile([128, NT, 128], F32R, name=f"wf{et}", tag="wf")
                nc.sync.dma_start(wt[:], d["d_wf"].ap()[l, et].bitcast(F32R))
                acc = yaccp.tile([128, S], F32, name=f"y{et}", tag="yacc")
                for dt in range(NT):
                    nc.tensor.matmul(acc[:], wt[:, dt, :], ybuf[:, dt, :],
                                     start=(dt == 0), stop=False)
                nc.tensor.matmul(acc[:], wesum[0:1, et * 128:(et + 1) * 128],
                                 negmu1[:], start=False, stop=True,
                                 skip_group_check=True)
                return acc

            def emit_ffn_evac(et, acc):
                nc.vector.tensor_tensor(ybuf2[:, et, :], acc[:], rstd1b[:],
                                        op=OP.mult)
                nc.scalar.activation(ybuf2[:, et, :], ybuf2[:, et, :],
                                     AF.Identity, bias=beffc[:, et:et + 1])
                sqt = sqpool.tile([128, S], BF16, name=f"sq2_{et}", tag=f"sq2_{et}")
                nc.vector.tensor_tensor(sqt[:], ybuf2[:, et, :].bitcast(F32),
                                        ybuf2[:, et, :].bitcast(F32), op=OP.mult)
                sq2[et] = sqt

            accs = [None] * NT
            accs[0] = emit_ffn_group(0)
            accs[1] = emit_ffn_group(1)
            # the squares-sum + rstd chain runs while FFN groups keep the
            # PE dense; the broadcast matmul lands before group 3 needs a
            # psum slot (its evac frees group 0's bank).
            for dt in range(NT):
                nc.tensor.matmul(s1t[32:33, :], ones_colb[:], sq1[dt][:],
                                 start=(dt == 0), stop=(dt == NT - 1))
            msq = spool.tile([1, S], F32, name="msq1", tag="ln_msq")
            nc.vector.tensor_scalar(msq[:], s1t[32:33, :], 1.0 / D, 1e-5, OP.mult, OP.add)
            musq = spool.tile([1, S], F32, name="musq1", tag="ln_musq")
            nc.vector.tensor_tensor(musq[:], negmu1[:].bitcast(F32),
                                    negmu1[:].bitcast(F32), op=OP.mult)
            var = spool.tile([1, S], F32, name="var1", tag="ln_var")
            nc.vector.tensor_tensor(var[:], msq[:], musq[:], op=OP.subtract)
            lnv = spool.tile([1, S], F32, name="lnv1", tag="ln_lnv")
            nc.scalar.activation(lnv[:], var[:], AF.Ln)
            rstd1 = spool.tile([1, S], F32R, name="rstd1", tag="ln_rstd")
            nc.scalar.activation(rstd1[:], lnv[:], AF.Exp, scale=-0.5)
            accs[2] = emit_ffn_group(2)
            rstd1_ps = lnp.tile([128, S], F32, name="rstd1_ps", tag="bc1")
            nc.tensor.matmul(rstd1_ps[:], one_row[:], rstd1[:], start=True, stop=True)
            nc.vector.tensor_copy(rstd1b[:], rstd1_ps[:])
            emit_ffn_evac(0, accs[0])
            for et in range(3, NT):
                accs[et] = emit_ffn_group(et)
                emit_ffn_evac(et - 2, accs[et - 2])
            emit_ffn_evac(NT - 2, accs[NT - 2])
            emit_ffn_evac(NT - 1, accs[NT - 1])

        # ---- LN2 -> xT (next layer input); beta folded downstream except
        # for the final layer (the kernel output needs the full affine).
        with (
            tc.tile_pool(name="ln2p", bufs=1, space="PSUM") as ln2p,
            tc.tile_pool(name="ln2rg", bufs=2, space="PSUM") as ln2rg,
        ):
            layernorm_fold(ybuf2, sq2, rows, 3 * D, 1e-5, xT, ln2p, ln2rg,
                           bias_col=fbc if l == L - 1 else None)

    # =======================================================================
    # output: transpose xT -> [S, D] and DMA out
    with (
        tc.tile_pool(name="out_sb", bufs=2) as outp,
        tc.tile_pool(name="out_ps", bufs=2, space="PSUM") as outps,
    ):
        for st in range(NST):
            ops_t = outps.tile([128, D], BF16, name="ops", tag="ops")
            for dt in range(NT):
                nc.tensor.transpose(ops_t[:, dt * 128:(dt + 1) * 128],
                                    xT[:, dt, st * 128:(st + 1) * 128],
                                    ident16[:])
            osb = outp.tile([128, D], F32, name="osb", tag="osb")
            nc.vector.tensor_copy(osb[:], ops_t[:])
            nc.sync.dma_start(d["d_out"].ap()[st * 128:(st + 1) * 128, :], osb[:])


# ---------------------------------------------------------------------------
def _pack_dd(w):
    """[L, Din, Dout] -> [L, et, p, n, e] with w[l, n*128+p, et*128+e]."""
    Lw = w.shape[0]
    return np.ascontiguousarray(
        w.reshape(Lw, NT, 128, NT, 128).transpose(0, 3, 2, 1, 4))


def kernel(**inputs):
    global _NC_CACHE
    if _NC_CACHE is None:
        _NC_CACHE = _build_nc()
    nc = _NC_CACHE

    import ml_dtypes
    f32 = lambda a: np.ascontiguousarray(np.asarray(a), dtype=np.float32)
    bf = lambda a: np.ascontiguousarray(a.astype(ml_dtypes.bfloat16))

    Wq = f32(inputs["Wq"]) * SCALE
    bq = f32(inputs["bq"]) * SCALE
    Wk = f32(inputs["Wk"])
    Wv = f32(inputs["Wv"])
    Wo = f32(inputs["Wo"])
    W1 = f32(inputs["W1"])
    W2 = f32(inputs["W2"])
    b1 = f32(inputs["b1"])
    b2 = f32(inputs["b2"])
    ag = f32(inputs["attn_ln_g"])
    ab = f32(inputs["attn_ln_b"])
    fg_ln = f32(inputs["ffn_ln_g"])
    fb_ln = f32(inputs["ffn_ln_b"])
    eb_ln = f32(inputs["emb_ln_b"])

    # collapse the (linear) FFN: y2 = W2 @ (W1 @ a + b1) + b2 with
    # a = g * z + b  =>  y2 = Weffg @ z + beff
    Weff = np.einsum("ldf,lfe->lde", W2, W1)          # [L, D, D]
    Weffg = Weff * ag[:, None, :]                      # fold LN1 gamma
    beff = (np.einsum("lde,le->ld", Weff, ab)
            + np.einsum("ldf,lf->ld", W2, b1) + b2)    # fold LN1 beta + biases
    wesum = Weffg.sum(axis=2)                          # [L, D]

    # fold the preceding LN's beta (emb LN for layer 0, LN2 of layer l-1
    # otherwise) into the QKV/residual biases: x_l = xT' + b_prev.
    b_prev = np.concatenate([eb_ln[None], fb_ln[:-1]], axis=0)   # [L, D]
    bq = bq + np.einsum("led,ld->le", Wq, b_prev)
    bk = f32(inputs["bk"]) + np.einsum("led,ld->le", Wk, b_prev)
    bv = f32(inputs["bv"]) + np.einsum("led,ld->le", Wv, b_prev)
    bo = f32(inputs["bo"]) + b_prev

    WqT = Wq.transpose(0, 2, 1)
    WkT = Wk.transpose(0, 2, 1)
    WvT = Wv.transpose(0, 2, 1)
    WoT = Wo.transpose(0, 2, 1)
    WfT = Weffg.transpose(0, 2, 1)

    WvA = WvT.reshape(L, NT, 128, D)                   # [l, n, p, e]
    wva = np.ascontiguousarray(WvA[:, :, :, 0:512].transpose(0, 2, 1, 3))
    wvb = np.ascontiguousarray(WvA[:, :, :, 512:768].transpose(0, 2, 1, 3))

    r = lambda a: a.reshape(L, NT, 128).transpose(0, 2, 1)  # [L,128,NT] cols
    cols = np.concatenate([r(bq), r(bk), r(beff), r(fg_ln), r(fb_ln)],
                          axis=2).astype(np.float32)   # [L, 128, 30]
    rows = np.concatenate([bv, bo, wesum, fg_ln],
                          axis=1).astype(np.float32)   # [L, 4*D]

    shared = {
        "wemb": f32(inputs["word_emb"]),
        "pemb": f32(inputs["pos_emb"])[:S],
        "temb": f32(inputs["type_emb"]),
        "egr": f32(inputs["emb_ln_g"]).reshape(1, D),
        "wq": bf(_pack_dd(WqT)),
        "wk": bf(_pack_dd(WkT)),
        "wo": bf(_pack_dd(WoT)),
        "wf": _pack_dd(WfT).astype(np.float32),
        "wva": bf(wva), "wvb": bf(wvb),
        "cols": np.ascontiguousarray(cols),
        "rows": np.ascontiguousarray(rows),
        "ones128": np.ones(128, np.float32),
        "ones128b": np.ones(128, ml_dtypes.bfloat16),
        "ones512": np.ones((1, 512), np.float32),
        "neg1": np.full((1, 128), -1.0, np.float32),
        "selA": np.concatenate([np.ones((1, 64)), np.zeros((1, 64))], 1).astype(np.float32),
        "selB": np.concatenate([np.zeros((1, 64)), np.ones((1, 64))], 1).astype(np.float32),
        "onesgridb": np.ones((128, NST * H), ml_dtypes.bfloat16),
    }
    ids = np.asarray(inputs["input_ids"]).astype(np.int32)
    tti = np.asarray(inputs["token_type_ids"]).astype(np.int32)
    am = np.asarray(inputs["attention_mask"]).astype(np.float32)
    in_maps = []
    for c in range(B):
        in_maps.append({
            **shared,
            "ids": ids[c].reshape(S, 1),
            "tti": tti[c].reshape(S, 1),
            "maskadd": np.where(am[c] == 0, -1e9, 0.0).astype(np.float32),
        })
    res = bass_utils.run_bass_kernel_spmd(
        nc, in_maps, core_ids=list(range(B)), trace=False)
    out = np.stack([res.results[c]["out"] for c in range(B)], axis=0)
    return out.astype(np.float32)


# revision 20
# speedup vs baseline: 1.2870x; 1.0096x over previous
"""BERT-base (12-layer, B=8, S=512, D=768, H=12, F=3072) forward pass on 8
Trainium2 NeuronCores — v2.

Strategy: data-parallel over batch (1 sequence per core, no collectives).
Key structure (per core, activations feature-major xT[D, S] in SBUF):
  - the FFN in this model is LINEAR (no activation between W1/W2), so
    W2@W1 collapses on the host into one 768x768 matrix Weff; the attn-LN
    affine (g,b) folds into Weff/beff as well, and the LN normalize
    (mean/rstd) folds into the GEMM via a rank-1 correction + per-column
    scale at PSUM evacuation. The FFN costs 1/4 of the naive FLOPs and
    layer-norm #1 never stalls the PE.
  - softmax denominators fall out of a ones-augmented V column (PV matmul
    M=65); reciprocals via Ln/Exp on the denominator row (partition 64).
  - weights are host-repacked so every weight DMA is contiguous per
    partition (1.5-6KB lines instead of 256B).
  - emission interleaves Q/K projections with scores+exp per head-pair so
    the ACT-bound softmax overlaps PE GEMM work; dummy warm matmuls keep
    the PE HAM clock at 2.4GHz across LN chains.
"""
import numpy as np

import concourse.bass as bass
import concourse.mybir as mybir
import concourse.tile as tile
from concourse import bass_utils
from concourse.masks import make_identity

AF = mybir.ActivationFunctionType
OP = mybir.AluOpType
F32 = mybir.dt.float32
F32R = mybir.dt.float32r
BF16 = mybir.dt.bfloat16
I32 = mybir.dt.int32

B, S, D, H, F, L, V = 8, 512, 768, 12, 3072, 12, 30522
DK = D // H
SCALE = 1.0 / float(np.sqrt(DK))
NT = D // 128      # 6 feature tiles
NST = S // 128     # 4 sequence tiles
NP = H // 2        # 6 head pairs

_NC_CACHE = None


# ---------------------------------------------------------------------------
# wait-slot legalization: walrus codegen allows only ONE sync-wait command on
# TPB instructions; hoist excess waits into standalone EventSemaphores.
def _legalize_waits(nc):
    skip = (mybir.InstEventSemaphore, mybir.InstNoOp)
    n = 0
    for fn in nc.m.functions:
        for blk in fn.blocks:
            out = []
            for inst in blk.instructions:
                si = inst.sync_info
                if si is not None and si.on_wait and not isinstance(inst, skip) \
                        and len(si.on_wait) > 1:
                    waits = list(si.on_wait)
                    for j, w in enumerate(waits[:-1]):
                        ev = mybir.InstEventSemaphore(
                            name=f"{inst.name}-lgw{j}", ins=[], outs=[],
                            sync_info=mybir.SyncInfo(on_wait=[w], on_update=[]),
                        )
                        ev.engine = inst.engine
                        out.append(ev)
                        n += 1
                    inst.sync_info = mybir.SyncInfo(
                        on_wait=[waits[-1]], on_update=list(si.on_update))
                out.append(inst)
            try:
                blk.instructions = out
            except Exception:
                blk.instructions.clear()
                blk.instructions.extend(out)
    return n


def _build_nc():
    nc = bass.Bass("TRN2", target_bir_lowering=False, debug=False,
                   enable_asserts=False, num_devices=8)

    # ---- DRAM I/O ---------------------------------------------------------
    d = {}
    d["d_ids"] = nc.dram_tensor("ids", [S, 1], I32, kind="ExternalInput")
    d["d_tti"] = nc.dram_tensor("tti", [S, 1], I32, kind="ExternalInput")
    d["d_mask"] = nc.dram_tensor("maskadd", [S], F32, kind="ExternalInput")
    d["d_wemb"] = nc.dram_tensor("wemb", [V, D], F32, kind="ExternalInput")
    d["d_pemb"] = nc.dram_tensor("pemb", [S, D], F32, kind="ExternalInput")
    d["d_temb"] = nc.dram_tensor("temb", [2, D], F32, kind="ExternalInput")

    # repacked weights: [L, et, p, n, e] so each (l, et) chunk is contiguous
    d["d_wq"] = nc.dram_tensor("wq", [L, NT, 128, NT, 128], BF16, kind="ExternalInput")
    d["d_wk"] = nc.dram_tensor("wk", [L, NT, 128, NT, 128], BF16, kind="ExternalInput")
    d["d_wo"] = nc.dram_tensor("wo", [L, NT, 128, NT, 128], BF16, kind="ExternalInput")
    d["d_wf"] = nc.dram_tensor("wf", [L, NT, 128, NT, 128], F32, kind="ExternalInput")
    d["d_wva"] = nc.dram_tensor("wva", [L, 128, NT, 512], BF16, kind="ExternalInput")
    d["d_wvb"] = nc.dram_tensor("wvb", [L, 128, NT, 256], BF16, kind="ExternalInput")
    # packed per-layer params: columns [128, 30] and rows [1, 3*768]
    d["d_cols"] = nc.dram_tensor("cols", [L, 128, 30], F32, kind="ExternalInput")
    d["d_rows"] = nc.dram_tensor("rows", [L, 4 * D], F32, kind="ExternalInput")
    d["d_egr"] = nc.dram_tensor("egr", [1, D], F32, kind="ExternalInput")
    d["d_ones"] = nc.dram_tensor("ones128", [128], F32, kind="ExternalInput")
    d["d_onesb"] = nc.dram_tensor("ones128b", [128], BF16, kind="ExternalInput")
    d["d_ones512"] = nc.dram_tensor("ones512", [1, 512], F32, kind="ExternalInput")
    d["d_neg1"] = nc.dram_tensor("neg1", [1, 128], F32, kind="ExternalInput")
    d["d_selA"] = nc.dram_tensor("selA", [1, 128], F32, kind="ExternalInput")
    d["d_selB"] = nc.dram_tensor("selB", [1, 128], F32, kind="ExternalInput")
    d["d_onesgb"] = nc.dram_tensor("onesgridb", [128, NST * H], BF16, kind="ExternalInput")
    d["d_out"] = nc.dram_tensor("out", [S, D], F32, kind="ExternalOutput")

    with tile.TileContext(nc) as tc:
        _emit(nc, tc, d)
    _legalize_waits(nc)
    return nc


def _emit(nc, tc, d):
    import contextlib
    ctx = contextlib.ExitStack()
    with ctx:
        _emit_body(nc, tc, d, ctx)


def _emit_body(nc, tc, d, ctx):
    pool = ctx.enter_context(tc.tile_pool(name="persist", bufs=1))
    wqp = ctx.enter_context(tc.tile_pool(name="wqp", bufs=2))
    wkp = ctx.enter_context(tc.tile_pool(name="wkp", bufs=2))
    wop = ctx.enter_context(tc.tile_pool(name="wop", bufs=4))
    wfp = ctx.enter_context(tc.tile_pool(name="wfp", bufs=4))
    vwp = ctx.enter_context(tc.tile_pool(name="vwp", bufs=2))
    ppool = ctx.enter_context(tc.tile_pool(name="params", bufs=2))
    rpool = ctx.enter_context(tc.tile_pool(name="rowsp", bufs=1))
    epool = ctx.enter_context(tc.tile_pool(name="epool", bufs=12))
    spool = ctx.enter_context(tc.tile_pool(name="smalls", bufs=1))
    sqpool = ctx.enter_context(tc.tile_pool(name="sqp", bufs=1))

    # ---- persistent constants --------------------------------------------
    ones_col = pool.tile([128, 1], F32R, name="ones_col")
    nc.sync.dma_start(ones_col[:], d["d_ones"].ap().rearrange("(p o) -> p o", o=1).bitcast(F32R))
    ones_colb = pool.tile([128, 1], BF16, name="ones_colb")
    nc.sync.dma_start(ones_colb[:], d["d_onesb"].ap().rearrange("(p o) -> p o", o=1))
    one_row = pool.tile([1, 128], F32R, name="one_row")
    nc.sync.dma_start(one_row[:], d["d_ones"].ap().rearrange("(o p) -> o p", o=1).bitcast(F32R))
    ones_s = pool.tile([1, 512], F32R, name="ones_s")
    nc.sync.dma_start(ones_s[:], d["d_ones512"].ap()[:, :].bitcast(F32R))
    neg_row = pool.tile([1, 128], F32R, name="neg_row")
    nc.sync.dma_start(neg_row[:], d["d_neg1"].ap()[:, :].bitcast(F32R))
    # head-select rows living on partition 64 (same partition as the PV
    # denominator row) so the broadcast matmul's operands share a base.
    sel64 = pool.tile([65, 2, 128], F32R, name="sel64")
    nc.sync.dma_start(sel64[64:65, 0, :], d["d_selA"].ap()[:, :].bitcast(F32R))
    nc.sync.dma_start(sel64[64:65, 1, :], d["d_selB"].ap()[:, :].bitcast(F32R))
    ident = pool.tile([128, 128], F32, name="ident")
    make_identity(nc, ident[:])
    ident16 = pool.tile([128, 128], BF16, name="ident16")
    make_identity(nc, ident16[:])
    maskc = pool.tile([128, NST], F32, name="maskc")
    nc.sync.dma_start(maskc[:], d["d_mask"].ap().rearrange("(n p) -> p n", p=128))

    # ---- persistent activations ------------------------------------------
    xT = pool.tile([128, NT, S], BF16, name="xT")       # layer input, feature-major
    qT = pool.tile([128, NT, S], BF16, name="qT")
    kT = pool.tile([128, NT, S], BF16, name="kT")
    cT = pool.tile([128, NT, S], BF16, name="cT")       # ctx, feature-major
    ybuf = pool.tile([128, NT, S], F32R, name="ybuf")   # post-Wo residual
    ybuf2 = pool.tile([128, NT, S], F32R, name="ybuf2")  # post-FFN (pre-LN2)
    vaug = pool.tile([128, NST, H, DK + 1], BF16, name="vaug")
    nc.sync.dma_start(
        vaug[:, :, :, DK:DK + 1],
        d["d_onesgb"].ap().rearrange("p (a b) -> p a b", a=NST)[:, :, :],
    )

    def warm_mm(wps):
        t = wps.tile([128, S], F32, name="warm", tag="warm")
        nc.tensor.matmul(t[:], one_row[:], ones_s[:], start=True, stop=True)

    # =======================================================================
    # folded layernorm (embedding LN + LN2): y [128, nt, S] F32R ->
    # out = (y - mu) * rstd * g  (gamma via rank-1 g x rstd broadcasts; beta
    # is host-folded into downstream biases, except `bias_col` for the last
    # layer). sq_t are precomputed squares of y. Dummy warm matmuls chained
    # on each output tile keep the PE HAM clock warm across the DVE tail.
    def layernorm_fold(y, sq_t, g_rows, g_off, eps, out, psum_pool, rgp,
                       bias_col=None):
        s0 = psum_pool.tile([1, S], F32, name="s0", tag="st0")
        s1t = psum_pool.tile([33, S], F32, name="s1t", tag="st1")
        # col-packed stats: s0 -> col group 0, s1 -> col group 1 (concurrent)
        for dt in range(NT):
            nc.tensor.matmul(s0[:], ones_col[:], y[:, dt, :],
                             start=(dt == 0), stop=(dt == NT - 1))
            nc.tensor.matmul(s1t[32:33, :], ones_colb[:], sq_t[dt][:],
                             start=(dt == 0), stop=(dt == NT - 1))
        mu = spool.tile([1, S], F32R, name="mu", tag="ln_mu")
        nc.vector.tensor_scalar(mu[:], s0[:], 1.0 / D, None, OP.mult)
        msq = spool.tile([1, S], F32, name="msq", tag="ln_msq")
        nc.vector.tensor_scalar(msq[:], s1t[32:33, :], 1.0 / D, eps, OP.mult, OP.add)
        musq = spool.tile([1, S], F32, name="musq", tag="ln_musq")
        nc.vector.tensor_tensor(musq[:], mu[:].bitcast(F32), mu[:].bitcast(F32), op=OP.mult)
        var = spool.tile([1, S], F32R, name="var", tag="ln_var")
        nc.vector.tensor_tensor(var[:], msq[:], musq[:], op=OP.subtract)
        # warm keeper chained on var: bridges the stats->rstd ACT stretch
        wv_ = psum_pool.tile([128, S], F32, name="wv", tag="warm")
        nc.tensor.matmul(wv_[:], one_row[:], var[:], start=True, stop=True)
        lnv = spool.tile([1, S], F32, name="lnv", tag="ln_lnv")
        nc.scalar.activation(lnv[:], var[:].bitcast(F32), AF.Ln)
        rstd = spool.tile([1, S], F32R, name="rstd", tag="ln_rstd")
        nc.scalar.activation(rstd[:], lnv[:], AF.Exp, scale=-0.5)
        negmu_ps = psum_pool.tile([128, S], F32, name="negmu_ps", tag="bc0")
        nc.tensor.matmul(negmu_ps[:], neg_row[:], mu[:], start=True, stop=True)
        nc.vector.tensor_tensor(y[:, 0, :], y[:, 0, :].bitcast(F32),
                                negmu_ps[:], op=OP.add)
        for dt in range(NT):
            rg = rgp.tile([128, S], F32, name=f"rg{dt}", tag=f"rg{dt % 2}")
            nc.tensor.matmul(rg[:], g_rows[:, g_off + dt * 128:g_off + (dt + 1) * 128],
                             rstd[:], start=True, stop=True)
            nc.vector.tensor_tensor(out[:, dt, :], y[:, dt, :].bitcast(F32),
                                    rg[:], op=OP.mult)
            if bias_col is not None:
                nc.scalar.activation(out[:, dt, :], out[:, dt, :],
                                     AF.Identity, bias=bias_col[:, dt:dt + 1])
            if dt + 1 < NT:
                nc.vector.tensor_tensor(y[:, dt + 1, :], y[:, dt + 1, :].bitcast(F32),
                                        negmu_ps[:], op=OP.add)
                # dep-chained warm keeper: waits on the tile just produced,
                # so it executes mid-tail instead of all-at-once.
                wt_ = psum_pool.tile([1, S], F32, name="wk", tag="warm")
                nc.tensor.matmul(wt_[:], ones_colb[:], out[:, dt, :],
                                 start=True, stop=True)

    # =======================================================================
    # embedding: gather + add + transpose to feature-major + LN -> xT
    egr = pool.tile([1, D], F32R, name="egr")
    nc.sync.dma_start(egr[:], d["d_egr"].ap()[:, :].bitcast(F32R))
    with (
        tc.tile_pool(name="emb_sb", bufs=3) as embp,
        tc.tile_pool(name="emb_ps", bufs=3, space="PSUM") as embps,
        tc.tile_pool(name="emb_wm", bufs=1, space="PSUM") as embwm,
    ):
        for st in range(NST):
            idst = embp.tile([128, 1], I32, name="idst", tag="idst")
            nc.sync.dma_start(idst[:], d["d_ids"].ap()[st * 128:(st + 1) * 128, :])
            ttst = embp.tile([128, 1], I32, name="ttst", tag="ttst")
            nc.sync.dma_start(ttst[:], d["d_tti"].ap()[st * 128:(st + 1) * 128, :])
            x0 = embp.tile([128, D], F32, name="x0", tag="x0")
            nc.gpsimd.indirect_dma_start(
                out=x0[:], out_offset=None, in_=d["d_wemb"].ap(),
                in_offset=bass.IndirectOffsetOnAxis(ap=idst[:, :1], axis=0))
            tg = embp.tile([128, D], F32, name="tg", tag="tg")
            nc.gpsimd.indirect_dma_start(
                out=tg[:], out_offset=None, in_=d["d_temb"].ap(),
                in_offset=bass.IndirectOffsetOnAxis(ap=ttst[:, :1], axis=0))
            pg = embp.tile([128, D], F32, name="pg", tag="pg")
            nc.sync.dma_start(pg[:], d["d_pemb"].ap()[st * 128:(st + 1) * 128, :])
            nc.vector.tensor_tensor(x0[:], x0[:], tg[:], op=OP.add)
            nc.vector.tensor_tensor(x0[:], x0[:], pg[:], op=OP.add)
            for dt in range(NT):
                trp = embps.tile([128, 128], F32, name="trp", tag="trp")
                nc.tensor.transpose(trp[:], x0[:, dt * 128:(dt + 1) * 128], ident[:])
                nc.vector.tensor_copy(ybuf[:, dt, st * 128:(st + 1) * 128], trp[:])
            # warm keeper chained on this chunk's transposed output
            wt_ = embwm.tile([1, 128], F32, name="ewk", tag="warm")
            nc.tensor.matmul(wt_[:], ones_col[:],
                             ybuf[:, NT - 1, st * 128:(st + 1) * 128],
                             start=True, stop=True)
    with (
        tc.tile_pool(name="eln_ps", bufs=1, space="PSUM") as elnps,
        tc.tile_pool(name="eln_rg", bufs=2, space="PSUM") as elnrg,
    ):
        sqe = []
        for dt in range(NT):
            sqt = sqpool.tile([128, S], BF16, name=f"sqe{dt}", tag=f"sq2_{dt}")
            nc.vector.tensor_tensor(sqt[:], ybuf[:, dt, :].bitcast(F32),
                                    ybuf[:, dt, :].bitcast(F32), op=OP.mult)
            sqe.append(sqt)
        layernorm_fold(ybuf, sqe, egr, 0, 1e-12, xT, elnps, elnrg)

    # =======================================================================
    # transformer layers
    for l in range(L):
        # ---- per-layer params (two packed DMAs) --------------------------
        colsc = ppool.tile([128, 30], F32, name="colsc", tag="colsc")
        nc.sync.dma_start(colsc[:], d["d_cols"].ap()[l])
        rows = rpool.tile([1, 4 * D], F32R, name="rows", tag="rows")
        nc.sync.dma_start(rows[:], d["d_rows"].ap()[l].rearrange("(o e) -> o e", o=1).bitcast(F32R))
        bqc = colsc[:, 0:6]
        bkc = colsc[:, 6:12]
        beffc = colsc[:, 12:18]
        fgc = colsc[:, 18:24]
        fbc = colsc[:, 24:30]
        bvr = rows[:, 0:D]
        bor = rows[:, D:2 * D]
        wesum = rows[:, 2 * D:3 * D]

        # ---- attention-scope psum pools ----------------------------------
        with (
            tc.tile_pool(name="accp", bufs=2, space="PSUM") as accp,
            tc.tile_pool(name="scp", bufs=2, space="PSUM") as scp,
            tc.tile_pool(name="ctxp", bufs=1, space="PSUM") as ctxp,
        ):
            # ---- V (seq-major, two column halves) ------------------------
            wva = vwp.tile([128, NT, 512], BF16, name="wva", tag="va")
            nc.sync.dma_start(wva[:], d["d_wva"].ap()[l])
            wvb = vwp.tile([128, NT, 256], BF16, name="wvb", tag="vb")
            nc.sync.dma_start(wvb[:], d["d_wvb"].ap()[l])
            for st in range(NST):
                acc = accp.tile([128, 512], F32, name=f"va{st}", tag="acc")
                for dt in range(NT):
                    nc.tensor.matmul(acc[:], xT[:, dt, st * 128:(st + 1) * 128],
                                     wva[:, dt, :], start=(dt == 0), stop=False)
                nc.tensor.matmul(acc[:], one_row[:], bvr[0:1, 0:512],
                                 start=False, stop=True, skip_group_check=True)
                nc.vector.tensor_copy(
                    vaug[:, st, 0:8, 0:DK],
                    acc[:].rearrange("p (a b) -> p a b", a=8))
            for st in range(NST):
                acc = accp.tile([128, 256], F32, name=f"vb{st}", tag="acc")
                for dt in range(NT):
                    nc.tensor.matmul(acc[:], xT[:, dt, st * 128:(st + 1) * 128],
                                     wvb[:, dt, :], start=(dt == 0), stop=False)
                nc.tensor.matmul(acc[:], one_row[:], bvr[0:1, 512:768],
                                 start=False, stop=True, skip_group_check=True)
                nc.vector.tensor_copy(
                    vaug[:, st, 8:12, 0:DK],
                    acc[:].rearrange("p (a b) -> p a b", a=4))

            # ---- attention machinery -------------------------------------
            e_tiles = [None] * NP

            def emit_qk(p):
                for (wpool_, wd, bcol, dst, nm) in (
                        (wqp, d["d_wq"], bqc, qT, "q"), (wkp, d["d_wk"], bkc, kT, "k")):
                    wt = wpool_.tile([128, NT, 128], BF16, name=f"w{nm}{p}", tag=f"w{nm}")
                    nc.sync.dma_start(wt[:], wd.ap()[l, p])
                    acc = accp.tile([128, S], F32, name=f"{nm}{p}", tag="acc")
                    for dt in range(NT):
                        nc.tensor.matmul(acc[:], wt[:, dt, :], xT[:, dt, :],
                                         start=(dt == 0), stop=(dt == NT - 1))
                    nc.vector.tensor_scalar(dst[:, p, :], acc[:],
                                            bcol[:, p:p + 1], None, OP.add)

            def emit_scores(p):
                ets = []
                for kt in range(NST):
                    sc = scp.tile([128, 2, S], F32, name=f"sc{kt}", tag="sc")
                    for hh in range(2):
                        lo, hi = hh * 64, hh * 64 + 64
                        nc.tensor.matmul(
                            sc[:, hh, :], kT[lo:hi, p, kt * 128:(kt + 1) * 128],
                            qT[lo:hi, p, :], start=True, stop=True)
                    et = epool.tile([128, 2, S], BF16, name=f"e{kt}", tag="e")
                    nc.scalar.activation(et[:, :, :], sc[:, :, :], AF.Exp,
                                         bias=maskc[:, kt:kt + 1])
                    ets.append(et)
                e_tiles[p] = ets

            def emit_pv(p):
                ets = e_tiles[p]
                cpss = []
                for hh in range(2):
                    h = 2 * p + hh
                    cps = ctxp.tile([DK + 1, S], F32, name=f"cps{hh}", tag=f"ctx{hh}")
                    for kt in range(NST):
                        nc.tensor.matmul(cps[:],
                                         vaug[:, kt, h, 0:DK + 1],
                                         ets[kt][:, hh, :],
                                         start=(kt == 0), stop=(kt == NST - 1))
                    cpss.append(cps)
                # reciprocal of the denominator row (partition 64)
                recips = []
                for hh in range(2):
                    nld = spool.tile([65, S], F32, name=f"nld{hh}", tag=f"nlden{hh}")
                    nc.scalar.activation(nld[64:65, :], cpss[hh][64:65, :], AF.Ln)
                    recip = spool.tile([65, S], F32R, name=f"rcp{hh}", tag=f"recip{hh}")
                    nc.scalar.activation(recip[64:65, :], nld[64:65, :], AF.Exp,
                                         scale=-1.0)
                    recips.append(recip)
                rpst = scp.tile([128, 2, S], F32, name="rps", tag="sc")
                rps = rpst[:, 0, :]
                nc.tensor.matmul(rps, sel64[64:65, 0, :], recips[0][64:65, :],
                                 start=True, stop=False)
                nc.tensor.matmul(rps, sel64[64:65, 1, :], recips[1][64:65, :],
                                 start=False, stop=True, skip_group_check=True)
                rsb = spool.tile([128, S], F32, name="rsb", tag="rsb")
                nc.vector.tensor_copy(rsb[:], rps)
                for hh in range(2):
                    lo = hh * 64
                    nc.vector.tensor_tensor(cT[lo:lo + DK, p, :], cpss[hh][0:DK, :],
                                            rsb[lo:lo + DK, :], op=OP.mult)

            # pipeline: Q/K + scores run ahead; PV trails by 2 pairs
            for p in range(NP):
                emit_qk(p)
                if p >= 2:
                    emit_pv(p - 2)
                emit_scores(p)
            emit_pv(NP - 2)
            emit_pv(NP - 1)

            # ---- Wo + residual -> ybuf; squares chase for LN1 ------------
            sq1 = [None] * NT
            for et in range(NT):
                wt = wop.tile([128, NT, 128], BF16, name=f"wo{et}", tag="wo")
                nc.sync.dma_start(wt[:], d["d_wo"].ap()[l, et])
                acc = accp.tile([128, S], F32, name=f"o{et}", tag="acc")
                for dt in range(NT):
                    nc.tensor.matmul(acc[:], wt[:, dt, :], cT[:, dt, :],
                                     start=(dt == 0), stop=False)
                nc.tensor.matmul(acc[:], bor[0:1, et * 128:(et + 1) * 128],
                                 ones_s[:], start=False, stop=True,
                                 skip_group_check=True)
                nc.vector.tensor_tensor(ybuf[:, et, :], acc[:],
                                        xT[:, et, :], op=OP.add)
                sqt = sqpool.tile([128, S], BF16, name=f"sq1_{et}", tag=f"sq1_{et}")
                nc.vector.tensor_tensor(sqt[:], ybuf[:, et, :].bitcast(F32),
                                        ybuf[:, et, :].bitcast(F32), op=OP.mult)
                sq1[et] = sqt

        # ---- LN1 (folded into Weff) + FFN --------------------------------
        with (
            tc.tile_pool(name="lnp", bufs=1, space="PSUM") as lnp,
            tc.tile_pool(name="yaccp", bufs=3, space="PSUM") as yaccp,
        ):
            s0 = lnp.tile([1, S], F32, name="s0", tag="st0")
            s1t = lnp.tile([33, S], F32, name="s1t", tag="st1")
            for dt in range(NT):
                nc.tensor.matmul(s0[:], ones_col[:], ybuf[:, dt, :],
                                 start=(dt == 0), stop=(dt == NT - 1))
            negmu1 = spool.tile([1, S], F32R, name="negmu1", tag="negmu1")
            nc.vector.tensor_scalar(negmu1[:], s0[:], -1.0 / D, None, OP.mult)

            # FFN: y2 = rstd1 .col* (Weffg @ ybuf - mu1 x wesum) + beff
            rstd1b = spool.tile([128, S], F32, name="rstd1b", tag="rstd1b")
            sq2 = [None] * NT

            def emit_ffn_group(et):
                wt = wfp.tile([128, NT, 128], F32R, name=f"wf{et}", tag="wf")
                nc.sync.dma_start(wt[:], d["d_wf"].ap()[l, et].bitcast(F32R))
                acc = yaccp.tile([128, S], F32, name=f"y{et}", tag="yacc")
                for dt in range(NT):
                    nc.tensor.matmul(acc[:], wt[:, dt, :], ybuf[:, dt, :],
                                     start=(dt == 0), stop=False)
                nc.tensor.matmul(acc[:], wesum[0:1, et * 128:(et + 1) * 128],
                                 negmu1[:], start=False, stop=True,
                                 skip_group_check=True)
                return acc

            def emit_ffn_evac(et, acc):
                nc.vector.tensor_tensor(ybuf2[:, et, :], acc[:], rstd1b[:],
                                        op=OP.mult)
                nc.scalar.activation(ybuf2[:, et, :], ybuf2[:, et, :],
                                     AF.Identity, bias=beffc[:, et:et + 1])
                sqt = sqpool.tile([128, S], BF16, name=f"sq2_{et}", tag=f"sq2_{et}")
                nc.vector.tensor_tensor(sqt[:], ybuf2[:, et, :].bitcast(F32),
                                        ybuf2[:, et, :].bitcast(F32), op=OP.mult)
                sq2[et] = sqt

            accs = [None] * NT
            accs[0] = emit_ffn_group(0)
            accs[1] = emit_ffn_group(1)
            # the squares-sum + rstd chain runs while FFN groups keep the
            # PE dense; the broadcast matmul lands before group 3 needs a
            # psum slot (its evac frees group 0's bank).
            for dt in range(NT):
                nc.tensor.matmul(s1t[32:33, :], ones_colb[:], sq1[dt][:],
                                 start=(dt == 0), stop=(dt == NT - 1))
            msq = spool.tile([1, S], F32, name="msq1", tag="ln_msq")
            nc.vector.tensor_scalar(msq[:], s1t[32:33, :], 1.0 / D, 1e-5, OP.mult, OP.add)
            musq = spool.tile([1, S], F32, name="musq1", tag="ln_musq")
            nc.vector.tensor_tensor(musq[:], negmu1[:].bitcast(F32),
                                    negmu1[:].bitcast(F32), op=OP.mult)
            var = spool.tile([1, S], F32, name="var1", tag="ln_var")
            nc.vector.tensor_tensor(var[:], msq[:], musq[:], op=OP.subtract)
            lnv = spool.tile([1, S], F32, name="lnv1", tag="ln_lnv")
            nc.scalar.activation(lnv[:], var[:], AF.Ln)
            rstd1 = spool.tile([1, S], F32R, name="rstd1", tag="ln_rstd")
            nc.scalar.activation(rstd1[:], lnv[:], AF.Exp, scale=-0.5)
            accs[2] = emit_ffn_group(2)
            rstd1_ps = lnp.tile([128, S], F32, name="rstd1_ps", tag="bc1")
            nc.tensor.matmul(rstd1_ps[:], one_row[:], rstd1[:], start=True, stop=True)
            nc.vector.tensor_copy(rstd1b[:], rstd1_ps[:])
            emit_ffn_evac(0, accs[0])
            for et in range(3, NT):
                accs[et] = emit_ffn_group(et)
                emit_ffn_evac(et - 2, accs[et - 2])
            emit_ffn_evac(NT - 2, accs[NT - 2])
            emit_ffn_evac(NT - 1, accs[NT - 1])

        # ---- LN2 -> xT (next layer input); beta folded downstream except
        # for the final layer (the kernel output needs the full affine).
        with (
            tc.tile_pool(name="ln2p", bufs=1, space="PSUM") as ln2p,
            tc.tile_pool(name="ln2rg", bufs=2, space="PSUM") as ln2rg,
        ):
            layernorm_fold(ybuf2, sq2, rows, 3 * D, 1e-5, xT, ln2p, ln2rg,
                           bias_col=fbc if l == L - 1 else None)

    # =======================================================================
    # output: transpose xT -> [S, D] and DMA out
    with (
        tc.tile_pool(name="out_sb", bufs=2) as outp,
        tc.tile_pool(name="out_ps", bufs=2, space="PSUM") as outps,
    ):
        for st in range(NST):
            ops_t = outps.tile([128, D], BF16, name="ops", tag="ops")
            for dt in range(NT):
                nc.tensor.transpose(ops_t[:, dt * 128:(dt + 1) * 128],
                                    xT[:, dt, st * 128:(st + 1) * 128],
                                    ident16[:])
            osb = outp.tile([128, D], F32, name="osb", tag="osb")
            nc.vector.tensor_copy(osb[:], ops_t[:])
            nc.sync.dma_start(d["d_out"].ap()[st * 128:(st + 1) * 128, :], osb[:])


# ---------------------------------------------------------------------------
def _pack_dd(w):
    """[L, Din, Dout] -> [L, et, p, n, e] with w[l, n*128+p, et*128+e]."""
    Lw = w.shape[0]
    return np.ascontiguousarray(
        w.reshape(Lw, NT, 128, NT, 128).transpose(0, 3, 2, 1, 4))


def kernel(**inputs):
    global _NC_CACHE
    if _NC_CACHE is None:
        _NC_CACHE = _build_nc()
    nc = _NC_CACHE

    import ml_dtypes
    f32 = lambda a: np.ascontiguousarray(np.asarray(a), dtype=np.float32)
    bf = lambda a: np.ascontiguousarray(a.astype(ml_dtypes.bfloat16))

    Wq = f32(inputs["Wq"]) * SCALE
    bq = f32(inputs["bq"]) * SCALE
    Wk = f32(inputs["Wk"])
    Wv = f32(inputs["Wv"])
    Wo = f32(inputs["Wo"])
    W1 = f32(inputs["W1"])
    W2 = f32(inputs["W2"])
    b1 = f32(inputs["b1"])
    b2 = f32(inputs["b2"])
    ag = f32(inputs["attn_ln_g"])
    ab = f32(inputs["attn_ln_b"])
    fg_ln = f32(inputs["ffn_ln_g"])
    fb_ln = f32(inputs["ffn_ln_b"])
    eb_ln = f32(inputs["emb_ln_b"])

    # collapse the (linear) FFN: y2 = W2 @ (W1 @ a + b1) + b2 with
    # a = g * z + b  =>  y2 = Weffg @ z + beff
    Weff = np.einsum("ldf,lfe->lde", W2, W1)          # [L, D, D]
    Weffg = Weff * ag[:, None, :]                      # fold LN1 gamma
    beff = (np.einsum("lde,le->ld", Weff, ab)
            + np.einsum("ldf,lf->ld", W2, b1) + b2)    # fold LN1 beta + biases
    wesum = Weffg.sum(axis=2)                          # [L, D]

    # fold the preceding LN's beta (emb LN for layer 0, LN2 of layer l-1
    # otherwise) into the QKV/residual biases: x_l = xT' + b_prev.
    b_prev = np.concatenate([eb_ln[None], fb_ln[:-1]], axis=0)   # [L, D]
    bq = bq + np.einsum("led,ld->le", Wq, b_prev)
    bk = f32(inputs["bk"]) + np.einsum("led,ld->le", Wk, b_prev)
    bv = f32(inputs["bv"]) + np.einsum("led,ld->le", Wv, b_prev)
    bo = f32(inputs["bo"]) + b_prev

    WqT = Wq.transpose(0, 2, 1)
    WkT = Wk.transpose(0, 2, 1)
    WvT = Wv.transpose(0, 2, 1)
    WoT = Wo.transpose(0, 2, 1)
    WfT = Weffg.transpose(0, 2, 1)

    WvA = WvT.reshape(L, NT, 128, D)                   # [l, n, p, e]
    wva = np.ascontiguousarray(WvA[:, :, :, 0:512].transpose(0, 2, 1, 3))
    wvb = np.ascontiguousarray(WvA[:, :, :, 512:768].transpose(0, 2, 1, 3))

    r = lambda a: a.reshape(L, NT, 128).transpose(0, 2, 1)  # [L,128,NT] cols
    cols = np.concatenate([r(bq), r(bk), r(beff), r(fg_ln), r(fb_ln)],
                          axis=2).astype(np.float32)   # [L, 128, 30]
    rows = np.concatenate([bv, bo, wesum, fg_ln],
                          axis=1).astype(np.float32)   # [L, 4*D]

    shared = {
        "wemb": f32(inputs["word_emb"]),
        "pemb": f32(inputs["pos_emb"])[:S],
        "temb": f32(inputs["type_emb"]),
        "egr": f32(inputs["emb_ln_g"]).reshape(1, D),
        "wq": bf(_pack_dd(WqT)),
        "wk": bf(_pack_dd(WkT)),
        "wo": bf(_pack_dd(WoT)),
        "wf": _pack_dd(WfT).astype(np.float32),
        "wva": bf(wva), "wvb": bf(wvb),
        "cols": np.ascontiguousarray(cols),
        "rows": np.ascontiguousarray(rows),
        "ones128": np.ones(128, np.float32),
        "ones128b": np.ones(128, ml_dtypes.bfloat16),
        "ones512": np.ones((1, 512), np.float32),
        "neg1": np.full((1, 128), -1.0, np.float32),
        "selA": np.concatenate([np.ones((1, 64)), np.zeros((1, 64))], 1).astype(np.float32),
        "selB": np.concatenate([np.zeros((1, 64)), np.ones((1, 64))], 1).astype(np.float32),
        "onesgridb": np.ones((128, NST * H), ml_dtypes.bfloat16),
    }
    ids = np.asarray(inputs["input_ids"]).astype(np.int32)
    tti = np.asarray(inputs["token_type_ids"]).astype(np.int32)
    am = np.asarray(inputs["attention_mask"]).astype(np.float32)
    in_maps = []
    for c in range(B):
        in_maps.append({
            **shared,
            "ids": ids[c].reshape(S, 1),
            "tti": tti[c].reshape(S, 1),
            "maskadd": np.where(am[c] == 0, -1e9, 0.0).astype(np.float32),
        })
    res = bass_utils.run_bass_kernel_spmd(
        nc, in_maps, core_ids=list(range(B)), trace=False)
    out = np.stack([res.results[c]["out"] for c in range(B)], axis=0)
    return out.astype(np.float32)
